# revision 1
# baseline (speedup 1.0000x reference)
"""Bass/Trainium2 kernel for nn_DirectionalGraphAttention (8 NeuronCores).

Math (see reference):
    q = (x@Wq.T + bq),  k = (x@Wk.T + bk),  v = (x@Wv.T + bv)      [N, C]
    scores[e,h] = q[row_e,h,:].k[col_e,h,:]/sqrt(HD) + ew_e
                  + (mean(x[col_e]) - mean(x[row_e])) * Wd[h] + bd[h]
    attn = softmax(scores, axis=0)            (global over ALL edges, per head)
    out[n,:] = (sum_{e: row_e==n} attn[e,h]*v[col_e,:]) @ Wo.T + bo

Strategy (8-way SPMD, one compiled program, per-core data differs):
  - Shard NODES into 8 contiguous ranges by destination; core r handles the
    edges whose row lands in its range (counts are ~E/8 by uniformity).
  - bd drops out (softmax over edges is invariant to per-head constants).
  - Each core builds the full kv table [N, 256] bf16 (k|v rows, 512B) in its
    DRAM via PE matmuls from a host-transposed x, then gathers per-edge rows
    with the dma_gather SWDGE ucode op. int16 gather indices limit tables to
    32768 rows, so edges are split into a "lo" pass (col < 32768) and a "hi"
    pass (col >= 32768), each sorted by destination row.
  - Destination rows are grouped into 128-node "slots". A compile-time
    schedule (max tile count over all cores, per phase+slot) keeps the SPMD
    program identical across cores; cores pad with dummy edges
    (ew = -1e9 -> exp = 0 -> zero contribution).
  - q[row] is never gathered: per tile of 128 edges, qexp = onehotT.T @ q_slot
    on the PE (one-hot matrices are built on host and shipped as bf16).
  - scores: DVE multiply (qexp*k) + segment reduce (16-wide heads), plus
    host-computed per-edge scalars ew and dmean (dmean = xm[col]-xm[row] from
    x row means).
  - Unnormalized msgs = v * exp(scores) are scatter-added with PE matmuls
    (msgs.T @ onehot) into PSUM, accumulated in SBUF [128c, Nloc].
  - The per-head softmax denominator is AllReduced (32 B) across the 8 cores;
    out_acc is scaled by 1/Z, multiplied by Wo.T on the PE and written out.
"""

import math
import os
import sys

sys.path.insert(0, "/opt/trn_rl_repo")

import numpy as np
import ml_dtypes

import concourse.bass as bass
import concourse.bacc as bacc
import concourse.mybir as mybir
import concourse.tile as tile
from concourse import bass_utils

BF16 = ml_dtypes.bfloat16

# ---------------------------------------------------------------- config ----
class Config:
    def __init__(self, N=50000, E=800000, n_cores=8, chunk_tiles=16,
                 tbl_split=32768):
        assert N % n_cores == 0
        self.N, self.E, self.R = N, E, n_cores
        self.C, self.H, self.HD = 128, 8, 16
        self.NLOC = N // n_cores                       # nodes per core
        self.NSLOT = -(-self.NLOC // 128)              # 128-node slots
        self.NLOCP = self.NSLOT * 128                  # padded local nodes
        self.NPAD = -(-N // 128) * 128                 # padded global nodes
        self.CT = chunk_tiles                          # tiles per chunk
        self.CE = chunk_tiles * 128                    # edges per chunk
        self.SPLIT = tbl_split                         # lo/hi table split
        assert self.SPLIT % 1 == 0 and self.SPLIT < 32768 + 1
        assert self.NPAD - self.SPLIT < 32768


FULL = Config()


# ------------------------------------------------------------- host prep ----
def _wrap16(idx):
    """int16 index vector [n] -> wrapped [128, n//16] layout for SWDGE ucode:
    index j is read from partition j%16, column j//16, replicated x8."""
    n = idx.shape[0]
    w = idx.reshape(n // 16, 16).T            # [16, n//16]
    return np.tile(w, (8, 1)).astype(np.int16)


def _host_prep(cfg, x, edge_index, edge_weight, Wd_vec):
    """Shard + schedule. Returns (sched, per_core) where sched is shared
    compile-time metadata and per_core is a list of input dicts."""
    N, E, R = cfg.N, cfg.E, cfg.R
    row = np.asarray(edge_index[0], dtype=np.int64)
    col = np.asarray(edge_index[1], dtype=np.int64)
    ew = np.asarray(edge_weight, dtype=np.float32)
    xnp = np.asarray(x, dtype=np.float32)
    xm = xnp.mean(axis=1)                              # [N] row means
    dm_all = (xm[col] - xm[row]).astype(np.float32)    # per-edge dmean

    # --- per-core edge lists: (phase, slot)-sorted ---
    core_of = row // cfg.NLOC
    per_core_edges = []          # [r] -> dict p -> dict s -> (cols, ews, dms, rel)
    counts = np.zeros((R, 2, cfg.NSLOT), dtype=np.int64)
    for r in range(R):
        m = core_of == r
        rl = row[m] - r * cfg.NLOC
        cl, wl, dl = col[m], ew[m], dm_all[m]
        phase = (cl >= cfg.SPLIT).astype(np.int64)     # 0 = lo, 1 = hi
        slot = rl // 128
        order = np.lexsort((rl, slot, phase))
        rl, cl, wl, dl, phase, slot = (a[order] for a in (rl, cl, wl, dl, phase, slot))
        buckets = {}
        for p in range(2):
            pm = phase == p
            buckets[p] = {}
            for s in range(cfg.NSLOT):
                sm = pm & (slot == s)
                cc = cl[sm] - (cfg.SPLIT if p else 0)
                buckets[p][s] = (cc, wl[sm], dl[sm], rl[sm] - s * 128)
                counts[r, p, s] = sm.sum()
        per_core_edges.append(buckets)

    # --- shared schedule: per (phase, slot) tile counts = max over cores ---
    ntile = np.maximum(1, -(-counts.max(axis=0) // 128))   # [2, NSLOT]
    # pad each phase to a multiple of CT tiles (append to last slot)
    for p in range(2):
        tp = int(ntile[p].sum())
        ntile[p, cfg.NSLOT - 1] += (-tp) % cfg.CT
    T_p = [int(ntile[p].sum()) for p in range(2)]
    T = sum(T_p)
    tile_slot = []                                     # [T] slot id
    tile_phase = []
    for p in range(2):
        for s in range(cfg.NSLOT):
            tile_slot += [s] * int(ntile[p, s])
            tile_phase += [p] * int(ntile[p, s])
    nchunk = T // cfg.CT
    chunk_phase = [tile_phase[c * cfg.CT] for c in range(nchunk)]
    for c in range(nchunk):   # a chunk must not mix lo/hi (one gather table)
        assert all(tile_phase[c * cfg.CT + t] == chunk_phase[c]
                   for t in range(cfg.CT))

    # start/stop flags for scatter psum accumulation per (phase, slot) run
    first_of_slot = [True] + [
        (tile_slot[i] != tile_slot[i - 1]) or (tile_phase[i] != tile_phase[i - 1])
        for i in range(1, T)]
    last_of_slot = first_of_slot[1:] + [True]

    sched = dict(T=T, nchunk=nchunk, tile_slot=tile_slot, tile_phase=tile_phase,
                 chunk_phase=chunk_phase, first=first_of_slot, last=last_of_slot)

    # --- weights (shared across cores) ---
    # (inputs come in as jax/np arrays)
    def f32(a):
        return np.asarray(a, dtype=np.float32)

    sched["has_biasA"] = None   # filled by caller with weights
    # --- per-core streams ---
    per_core = []
    for r in range(R):
        cols = np.zeros(T * 128, dtype=np.int16)
        ews = np.full(T * 128, -1e9, dtype=np.float32)
        dms = np.zeros(T * 128, dtype=np.float32)
        rels = np.zeros(T * 128, dtype=np.int64)
        pos = 0
        for p in range(2):
            for s in range(cfg.NSLOT):
                cc, wl, dl, rl = per_core_edges[r][p][s]
                n = len(cc)
                room = int(ntile[p, s]) * 128
                assert n <= room
                cols[pos:pos + n] = cc.astype(np.int16)
                ews[pos:pos + n] = wl
                dms[pos:pos + n] = dl
                rels[pos:pos + n] = rl
                pos += room
        assert pos == T * 128

        # one-hot matrices [tile, e, w] and transpose, chunked
        oh = np.zeros((T, 128, 128), dtype=ml_dtypes.float8_e4m3)
        ti = np.repeat(np.arange(T), 128)
        ei = np.tile(np.arange(128), T)
        oh[ti, ei, rels] = 1
        oh_c = (oh.reshape(nchunk, cfg.CT, 128, 128)
                  .transpose(0, 2, 1, 3).reshape(nchunk, 128, cfg.CE))
        ohT_c = (oh.transpose(0, 2, 1).reshape(nchunk, cfg.CT, 128, 128)
                   .transpose(0, 2, 1, 3).reshape(nchunk, 128, cfg.CE))
        colidx = np.stack([_wrap16(cols[c * cfg.CE:(c + 1) * cfg.CE])
                           for c in range(nchunk)])    # [nchunk, 128, CE//16]
        # per-edge per-head score bias: ew + dmean*Wd  [T*128, H] f32
        bias_eh = (ews[:, None] + dms[:, None] * Wd_vec[None, :]).astype(
            np.float32)
        CT = cfg.CT
        metas = []
        for c in range(nchunk):
            # layout [128(e), CT, H]: partition = edge-within-tile
            bb = bias_eh[c * cfg.CE:(c + 1) * cfg.CE]
            bb = bb.reshape(CT, 128, cfg.H).transpose(1, 0, 2).reshape(
                128, CT * cfg.H)
            bias_b = np.ascontiguousarray(bb).view(np.uint8)
            idx_b = np.ascontiguousarray(colidx[c]).view(np.uint8)
            metas.append(np.concatenate([bias_b, idx_b], axis=1))
        meta = np.stack(metas)      # [nchunk, 128, CT*H*4 + CE//8]

        per_core.append(dict(
            meta=np.ascontiguousarray(meta),
            onehot=np.ascontiguousarray(oh_c),
            onehotT=np.ascontiguousarray(ohT_c),
            colidx=np.ascontiguousarray(colidx),   # kept for emu/debug only
        ))
    return sched, per_core


# ---------------------------------------------------------- kernel build ----
def _build(nc, cfg, sched, has_biasA, has_biasB, has_bo, debug_taps=False):
    f32, bf16, i16 = mybir.dt.float32, mybir.dt.bfloat16, mybir.dt.int16
    fp8 = mybir.dt.float8e4
    C = cfg.C
    NT_G = cfg.NPAD // 128       # global projection tiles
    NS = cfg.NSLOT
    T, nchunk = sched["T"], sched["nchunk"]
    NBLK = -(-NS // 4)           # 512-col psum blocks over slots

    # ---- I/O ----
    xT = nc.dram_tensor("xT", [128, cfg.NPAD], bf16, kind="ExternalInput").ap()
    xTloc = nc.dram_tensor("xTloc", [128, cfg.NLOCP], bf16,
                           kind="ExternalInput").ap()
    W_A = nc.dram_tensor("W_A", [128, 2 * C], bf16, kind="ExternalInput").ap()
    biasA = nc.dram_tensor("biasA", [128, 2 * C], f32, kind="ExternalInput").ap()
    W_B = nc.dram_tensor("W_B", [128, C], bf16, kind="ExternalInput").ap()
    biasB = nc.dram_tensor("biasB", [128, C], f32, kind="ExternalInput").ap()
    WoT = nc.dram_tensor("WoT", [128, C], f32, kind="ExternalInput").ap()
    bo_r = nc.dram_tensor("bo_r", [128, C], f32, kind="ExternalInput").ap()
    ones_c = nc.dram_tensor("ones_c", [128, 1], f32, kind="ExternalInput").ap()
    Mrep = nc.dram_tensor("Mrep", [cfg.H, 128], f32, kind="ExternalInput").ap()
    MW = cfg.CT * cfg.H * 4 + cfg.CE // 8   # meta bytes/partition: bias|colidx
    meta_d = nc.dram_tensor("meta", [nchunk, 128, MW], mybir.dt.uint8,
                            kind="ExternalInput").ap()
    onehot_i = nc.dram_tensor("onehot", [nchunk, 128, cfg.CE], fp8,
                              kind="ExternalInput").ap()
    onehotT_i = nc.dram_tensor("onehotT", [nchunk, 128, cfg.CE], fp8,
                               kind="ExternalInput").ap()
    out = nc.dram_tensor("out", [cfg.NLOC, C], f32, kind="ExternalOutput").ap()
    if debug_taps:
        dbg_q = nc.dram_tensor("dbg_q", [128, NS * 128], f32,
                               kind="ExternalOutput").ap()
        dbg_z = nc.dram_tensor("dbg_z", [128, cfg.H], f32,
                               kind="ExternalOutput").ap()
        dbg_acc = nc.dram_tensor("dbg_acc", [128, NS * 128], f32,
                                 kind="ExternalOutput").ap()
        dbg_sc = nc.dram_tensor("dbg_sc", [128, cfg.CT * cfg.H], f32,
                                kind="ExternalOutput").ap()
        dbg_kv = nc.dram_tensor("dbg_kv", [128, cfg.CT * 2 * C], f32,
                                kind="ExternalOutput").ap()
        dbg_ms = nc.dram_tensor("dbg_ms", [128, cfg.CT * C], f32,
                                kind="ExternalOutput").ap()
        dbg_fl = nc.dram_tensor("dbg_fl", [8, 128, 512], f32,
                                kind="ExternalOutput").ap()

    with tile.TileContext(nc) as tc:
        with (
            tc.tile_pool(name="persist", bufs=1) as pp,
            tc.tile_pool(name="wpool", bufs=1) as wp,
            tc.tile_pool(name="io", bufs=3) as iop,
            tc.tile_pool(name="psA", bufs=2, space="PSUM") as psA,
            tc.tile_pool(name="psS", bufs=2, space="PSUM") as psS,
            tc.tile_pool(name="work", bufs=3) as wk,
            tc.tile_pool(name="dram", bufs=1, space="DRAM") as dp,
        ):
            # persistent SBUF
            qlocal = pp.tile([128, NS * 128], bf16, tag="qlocal")
            out_acc = pp.tile([128, NS * 128], f32, tag="out_acc")
            zacc = pp.tile([128, cfg.H], f32, tag="zacc")
            nc.vector.memset(zacc[:], 0.0)

            # weights in SBUF
            WA_sb = wp.tile([128, 2 * C], bf16, tag="WA")
            nc.sync.dma_start(WA_sb[:], W_A[:])
            WB_sb = wp.tile([128, C], bf16, tag="WB")
            nc.sync.dma_start(WB_sb[:], W_B[:])
            WoT_sb = wp.tile([128, C], f32, tag="WoT")
            nc.sync.dma_start(WoT_sb[:], WoT[:])
            bo_sb = wp.tile([128, C], f32, tag="bo")
            nc.sync.dma_start(bo_sb[:], bo_r[:])
            ones_sb = wp.tile([128, 1], f32, tag="ones")
            nc.sync.dma_start(ones_sb[:], ones_c[:])
            Mrep_sb = wp.tile([cfg.H, 128], f32, tag="Mrep")
            nc.sync.dma_start(Mrep_sb[:], Mrep[:])
            if has_biasA:
                bA_sb = wp.tile([128, 2 * C], f32, tag="bA")
                nc.sync.dma_start(bA_sb[:], biasA[:])
            if has_biasB:
                bB_sb = wp.tile([128, C], f32, tag="bB")
                nc.sync.dma_start(bB_sb[:], biasB[:])

            # DRAM kv tables (split so lo-phase gathers only depend on
            # the first part of P1)
            kv_tab_lo = dp.tile([cfg.SPLIT, 2 * C], bf16)
            kv_tab_hi = dp.tile([cfg.NPAD - cfg.SPLIT, 2 * C], bf16)

            # ------------- P2: local q tilde -> DRAM q table -------------
            s = 0
            while s < NS:
                nb = min(4, NS - s)
                xt4 = iop.tile([128, 512], bf16, tag="xt2")
                nc.scalar.dma_start(xt4[:, 0:nb * 128],
                                    xTloc[:, s * 128:(s + nb) * 128])
                for j in range(nb):
                    ps = psA.tile([128, 2 * C], f32, tag="psA")
                    nc.tensor.matmul(out=ps[:, 0:C],
                                     lhsT=xt4[:, j * 128:(j + 1) * 128],
                                     rhs=WB_sb[:], start=True, stop=True)
                    dstq = qlocal[:, (s + j) * 128:(s + j + 1) * 128]
                    if has_biasB:
                        nc.vector.tensor_tensor(out=dstq, in0=ps[:, 0:C],
                                                in1=bB_sb[:],
                                                op=mybir.AluOpType.add)
                    elif (s + j) % 2 == 0:
                        nc.scalar.copy(dstq, ps[:, 0:C])
                    else:
                        nc.vector.tensor_copy(dstq, ps[:, 0:C])
                s += nb

            # ---------------- P1: kv table (4 tiles per DMA) ----------------
            gt = 0
            while gt < NT_G:
                nb = min(4, NT_G - gt)
                xt4 = iop.tile([128, 512], bf16, tag="xt")
                nc.sync.dma_start(xt4[:, 0:nb * 128],
                                  xT[:, gt * 128:(gt + nb) * 128])
                kv4 = iop.tile([128, 4, 2 * C], bf16, tag="kv4")
                for j2 in range(0, nb, 2):
                    nj = min(2, nb - j2)
                    ps = psA.tile([128, 2 * 2 * C], f32, tag="psA")
                    for j in range(j2, j2 + nj):
                        nc.tensor.matmul(out=ps[:, (j - j2) * 2 * C:
                                                (j - j2 + 1) * 2 * C],
                                         lhsT=xt4[:, j * 128:(j + 1) * 128],
                                         rhs=WA_sb[:], start=True, stop=True)
                    src_ap = ps[:, 0:nj * 2 * C].rearrange(
                        "p (j c) -> p j c", j=nj)
                    if has_biasA:
                        nc.vector.tensor_tensor(
                            out=kv4[:, j2:j2 + nj, :], in0=src_ap,
                            in1=bA_sb[:].rearrange("p c -> p () c")
                                        .to_broadcast([128, nj, 2 * C]),
                            op=mybir.AluOpType.add)
                    elif (gt + j2) % 4 == 0:
                        nc.scalar.copy(kv4[:, j2:j2 + nj, :], src_ap)
                    else:
                        nc.vector.tensor_copy(kv4[:, j2:j2 + nj, :], src_ap)
                r0 = gt * 128
                if r0 < cfg.SPLIT:
                    assert (gt + nb) * 128 <= cfg.SPLIT
                    dst = kv_tab_lo[r0:r0 + nb * 128, :]
                else:
                    dst = kv_tab_hi[r0 - cfg.SPLIT:r0 - cfg.SPLIT + nb * 128, :]
                eng = nc.sync if (gt // 4) % 2 == 0 else nc.scalar
                eng.dma_start(
                    dst.rearrange("(j p) c -> p j c", j=nb, p=128),
                    kv4[:, 0:nb, :])
                gt += nb

            # ---------------- P3: edge chunks ----------------
            CT, CE = cfg.CT, cfg.CE
            ngroup = CT // 4
            kv_lo = kv_tab_lo[:]
            kv_hi = kv_tab_hi[:]
            scat_ps = None          # current scatter psum bank
            cur_blk = -1

            def flush_block(blk, ps_tile):
                lo, hi_ = blk * 4, min(blk * 4 + 4, NS)
                w = (hi_ - lo) * 128
                if debug_taps and sched["fl_n"] < 8:
                    flt = wk.tile([128, 512], f32, tag="flt")
                    nc.vector.tensor_copy(flt[:], ps_tile[:])
                    nc.sync.dma_start(dbg_fl[sched["fl_n"], :, :], flt[:])
                    sched["fl_n"] += 1
                dst = out_acc[:, blk * 512: blk * 512 + w]
                # (scatter matmuls accumulate with start=False into a
                # pre-zeroed bank: see memset at allocation)
                if sched["blk_seen"][blk]:
                    nc.vector.tensor_tensor(out=dst, in0=dst, in1=ps_tile[:, 0:w],
                                            op=mybir.AluOpType.add)
                else:
                    nc.vector.tensor_copy(dst, ps_tile[:, 0:w])
                    sched["blk_seen"][blk] = True

            sched["blk_seen"] = [False] * NBLK
            sched["fl_n"] = 0

            for c in range(nchunk):
                tab = kv_hi if sched["chunk_phase"][c] else kv_lo
                MB = CT * cfg.H * 4
                meta_sb = wk.tile([128, MB + CE // 8], mybir.dt.uint8,
                                  tag="meta")
                nc.scalar.dma_start(meta_sb[:], meta_d[c, :, :])
                bias_sb = meta_sb[:, 0:MB].bitcast(f32)
                idx_sb = meta_sb[:, MB:].bitcast(i16)
                kv_g = wk.tile([128, CT, 2 * C], bf16, tag="kv_g")
                # The SWDGE descriptor ring caps one call at ~1024 descs
                # (2048 hangs the ucode); split the chunk gather.
                GSUB = 1024
                for g2 in range(-(-CE // GSUB)):
                    e0, e1 = g2 * GSUB, min((g2 + 1) * GSUB, CE)
                    nc.gpsimd.dma_gather(
                        out_ap=kv_g[:, e0 // 128:e1 // 128, :], in_ap=tab,
                        idxs_ap=idx_sb[:, e0 // 16:e1 // 16],
                        num_idxs=e1 - e0, num_idxs_reg=e1 - e0,
                        elem_size=2 * C, queue_num=g2 % 4)
                oh_sb = wk.tile([128, CE], fp8, tag="oh")
                nc.scalar.dma_start(oh_sb[:], onehot_i[c, :, :])
                ohT_sb = wk.tile([128, CE], fp8, tag="ohT")
                nc.scalar.dma_start(ohT_sb[:], onehotT_i[c, :, :])

                scores = wk.tile([128, CT * cfg.H], f32, tag="scores")
                qbf = wk.tile([128, CT * C], bf16, tag="qbf")
                for g in range(ngroup):
                    qps = psS.tile([128, 512], f32, tag="qexp")
                    for j in range(4):
                        t = g * 4 + j
                        sl = sched["tile_slot"][c * CT + t]
                        nc.tensor.matmul(
                            out=qps[:, j * 128:(j + 1) * 128],
                            lhsT=ohT_sb[:, t * 128:(t + 1) * 128],
                            rhs=qlocal[:, sl * 128:(sl + 1) * 128],
                            start=True, stop=True)
                    nc.scalar.copy(qbf[:, g * 512:(g + 1) * 512], qps[:])
                prod = wk.tile([128, CT * C], bf16, tag="prod")
                nc.vector.tensor_tensor(
                    out=prod[:].rearrange("p (t c) -> p t c", t=CT, c=C),
                    in0=qbf[:].rearrange("p (t c) -> p t c", t=CT, c=C),
                    in1=kv_g[:, :, 0:C],
                    op=mybir.AluOpType.mult)
                nc.vector.tensor_reduce(
                    out=scores[:],
                    in_=prod[:].rearrange("p (t h d) -> p t h d",
                                          t=CT, h=cfg.H, d=cfg.HD),
                    axis=mybir.AxisListType.X, op=mybir.AluOpType.add)

                # scores += ew + dmean*Wd (host-precomputed per-edge bias)
                nc.vector.tensor_tensor(out=scores[:], in0=scores[:],
                                        in1=bias_sb, op=mybir.AluOpType.add)

                if debug_taps and c == 0:
                    nc.sync.dma_start(dbg_sc[:], scores[:])
                    kvf = wk.tile([128, CT * 2 * C], f32, tag="kvf")
                    nc.vector.tensor_copy(kvf[:], kv_g[:].rearrange(
                        "p t c -> p (t c)"))
                    nc.sync.dma_start(dbg_kv[:], kvf[:])
                exps = wk.tile([128, CT * cfg.H], bf16, tag="exps")
                nc.scalar.activation(exps[:], scores[:],
                                     mybir.ActivationFunctionType.Exp)
                ztmp = wk.tile([128, cfg.H], f32, tag="ztmp")
                nc.vector.tensor_reduce(
                    out=ztmp[:],
                    in_=exps[:].rearrange("p (t h) -> p h t", t=CT, h=cfg.H),
                    axis=mybir.AxisListType.X, op=mybir.AluOpType.add)
                nc.vector.tensor_tensor(out=zacc[:], in0=zacc[:], in1=ztmp[:],
                                        op=mybir.AluOpType.add)

                msgs = wk.tile([128, CT * C], bf16, tag="msgs")
                nc.vector.tensor_tensor(
                    out=msgs[:].rearrange("p (t h d) -> p t h d",
                                          t=CT, h=cfg.H, d=cfg.HD),
                    in0=kv_g[:, :, C:2 * C].rearrange(
                        "p t (h d) -> p t h d", h=cfg.H, d=cfg.HD),
                    in1=exps[:].rearrange("p (t h) -> p t h ()",
                                          t=CT, h=cfg.H)
                               .to_broadcast([128, CT, cfg.H, cfg.HD]),
                    op=mybir.AluOpType.mult)

                if debug_taps and c == 0:
                    msf = wk.tile([128, CT * C], f32, tag="msf")
                    nc.vector.tensor_copy(msf[:], msgs[:])
                    nc.sync.dma_start(dbg_ms[:], msf[:])
                for t in range(CT):
                    gt = c * CT + t
                    s = sched["tile_slot"][gt]
                    blk = s // 4
                    if blk != cur_blk:
                        if scat_ps is not None:
                            flush_block(cur_blk, scat_ps)
                        scat_ps = psS.tile([128, 512], f32, tag="scat")
                        # PSUM accumulation groups cannot survive same-bank
                        # interleaving (HW): pre-zero the bank and use
                        # start=False on every matmul instead.
                        nc.vector.memset(scat_ps[:], 0.0)
                        cur_blk = blk
                    nc.tensor.matmul(
                        out=scat_ps[:, (s % 4) * 128:(s % 4) * 128 + 128],
                        lhsT=msgs[:, t * C:(t + 1) * C],
                        rhs=oh_sb[:, t * 128:(t + 1) * 128],
                        start=False, stop=True, skip_group_check=True)
            flush_block(cur_blk, scat_ps)

            # ---------------- P4: finale ----------------
            if debug_taps:
                nc.sync.dma_start(dbg_z[:], zacc[:])
                nc.sync.dma_start(dbg_acc[:], out_acc[:])
            zsum_ps = psA.tile([128, 2 * C], f32, tag="psA")
            nc.tensor.matmul(out=zsum_ps[0:1, 0:cfg.H], lhsT=ones_sb[:],
                             rhs=zacc[:], start=True, stop=True)
            zsb = wk.tile([1, cfg.H], f32, tag="zsb")
            nc.vector.tensor_copy(zsb[:], zsum_ps[0:1, 0:cfg.H])
            zin_d = dp.tile([1, cfg.H], f32)
            zout_d = dp.tile([1, cfg.H], f32)
            nc.sync.dma_start(zin_d[:], zsb[:])
            nc.gpsimd.collective_compute(
                "AllReduce", mybir.AluOpType.add,
                replica_groups=[list(range(cfg.R))],
                ins=[zin_d.opt()], outs=[zout_d.opt()])
            zvec = wk.tile([cfg.H, 1], f32, tag="zvec")
            nc.sync.dma_start(zvec[:], zout_d[:].rearrange("a h -> h a"))
            zcol_ps = psA.tile([128, 2 * C], f32, tag="psA")
            nc.tensor.matmul(out=zcol_ps[:, 0:1], lhsT=Mrep_sb[:], rhs=zvec[:],
                             start=True, stop=True)
            rz = wk.tile([128, 1], f32, tag="rz")
            nc.vector.reciprocal(rz[:], zcol_ps[:, 0:1])
            nc.vector.tensor_scalar(out=out_acc[:], in0=out_acc[:],
                                    scalar1=rz[:], scalar2=None,
                                    op0=mybir.AluOpType.mult)

            for s in range(NS):
                rows = min(128, cfg.NLOC - s * 128)
                ps = psA.tile([128, 2 * C], f32, tag="psA")
                ps = ps[:, 0:C]
                nc.tensor.matmul(out=ps,
                                 lhsT=out_acc[:, s * 128:(s + 1) * 128],
                                 rhs=WoT_sb[:], start=True, stop=True)
                of = iop.tile([128, C], f32, tag="of")
                if has_bo:
                    nc.vector.tensor_tensor(out=of[:], in0=ps, in1=bo_sb[:],
                                            op=mybir.AluOpType.add)
                else:
                    nc.vector.tensor_copy(of[:], ps)
                nc.sync.dma_start(out[s * 128:s * 128 + rows, :], of[0:rows, :])
    return nc


# -------------------------------------------------------------- frontend ----
def _run(cfg, inputs, trace=False):
    x = np.asarray(inputs["x"], dtype=np.float32)
    sched, per_core = _host_prep(cfg, x, inputs["edge_index"],
                                 inputs["edge_weight"],
                                 np.asarray(inputs["Wd"],
                                            np.float32).reshape(-1))

    f32 = np.float32
    Wq = np.asarray(inputs["Wq"], f32); bq = np.asarray(inputs["bq"], f32)
    Wk = np.asarray(inputs["Wk"], f32); bk = np.asarray(inputs["bk"], f32)
    Wv = np.asarray(inputs["Wv"], f32); bv = np.asarray(inputs["bv"], f32)
    Wd = np.asarray(inputs["Wd"], f32).reshape(-1)
    Wo = np.asarray(inputs["Wo"], f32); bo = np.asarray(inputs["bo"], f32)
    inv = 1.0 / math.sqrt(cfg.HD)

    xT_pad = np.zeros((128, cfg.NPAD), BF16)
    xT_pad[:, :cfg.N] = x.T.astype(BF16)
    W_A = np.concatenate([Wk.T, Wv.T], axis=1).astype(BF16)   # [128, 256]
    biasA = np.tile(np.concatenate([bk, bv])[None, :], (128, 1))
    W_B = (Wq.T * inv).astype(BF16)
    biasB = np.tile((bq * inv)[None, :], (128, 1))
    has_biasA = bool(np.any(biasA)); has_biasB = bool(np.any(biasB))
    has_bo = bool(np.any(bo))
    Mrep = np.zeros((cfg.H, 128), f32)
    for h in range(cfg.H):
        Mrep[h, h * 16:(h + 1) * 16] = 1.0

    base = dict(
        xT=xT_pad, W_A=W_A, biasA=biasA.astype(f32), W_B=W_B,
        biasB=biasB.astype(f32), WoT=np.ascontiguousarray(Wo.T),
        bo_r=np.tile(bo[None, :], (128, 1)).astype(f32),
        ones_c=np.ones((128, 1), f32), Mrep=Mrep)

    in_maps = []
    for r in range(cfg.R):
        xloc = np.zeros((128, cfg.NLOCP), BF16)
        xloc[:, :cfg.NLOC] = x[r * cfg.NLOC:(r + 1) * cfg.NLOC].T.astype(BF16)
        m = dict(base)
        m["xTloc"] = xloc
        m.update(per_core[r])
        in_maps.append(m)

    nc = bacc.Bacc("TRN2", target_bir_lowering=False, debug=False,
                   num_devices=cfg.R, num_swdge_queues=4)
    _build(nc, cfg, sched, has_biasA, has_biasB, has_bo)
    nc.compile()

    res = bass_utils.run_bass_kernel_spmd(
        nc, in_maps, core_ids=list(range(cfg.R)), trace=trace)
    outs = [res.results[r]["out"] for r in range(cfg.R)]
    full = np.concatenate(outs, axis=0).astype(np.float32)
    return full, res


def kernel(**inputs):
    out, _ = _run(FULL, inputs)
    return out


if __name__ == "__main__":
    pass



# revision 4
# speedup vs baseline: 1.4441x; 1.4441x over previous
"""Bass/Trainium2 kernel for nn_DirectionalGraphAttention (8 NeuronCores).

Math (see reference):
    q = (x@Wq.T + bq),  k = (x@Wk.T + bk),  v = (x@Wv.T + bv)      [N, C]
    scores[e,h] = q[row_e,h,:].k[col_e,h,:]/sqrt(HD) + ew_e
                  + (mean(x[col_e]) - mean(x[row_e])) * Wd[h] + bd[h]
    attn = softmax(scores, axis=0)            (global over ALL edges, per head)
    out[n,:] = (sum_{e: row_e==n} attn[e,h]*v[col_e,:]) @ Wo.T + bo

Strategy (8-way SPMD, one compiled program, per-core data differs):
  - Shard NODES into 8 contiguous ranges by destination; core r handles the
    edges whose row lands in its range (counts are ~E/8 by uniformity).
  - bd drops out (softmax over edges is invariant to per-head constants).
  - Per-edge source features are HOST-pregathered: for each 128-edge tile the
    host ships xgT[ch, e] = x[col_e, ch].T in bf16, packed together with the
    per-edge score bias (ew + dmean*Wd), the scatter one-hot and its
    transpose into ONE per-chunk DMA stream. No DRAM kv table, no SWDGE
    gather (the v1 design spent ~0.5 ms/core in gather descriptor ucode and
    ~5 ms of DMA-engine-seconds on 512 B gathered rows).
  - k|v are computed on the fly per tile: kv[e, 0:256] = xgT_tile.T @
    [Wk.T|Wv.T] — one 256-wide PE matmul per tile (FWL weight loads), output
    kept in PSUM (f32; skips v1's bf16 table rounding).
  - Destination rows are grouped into 128-node "slots" (sorted, padded to a
    shared compile-time schedule; pad edges have ew=-1e9 -> exp=0).
  - q[row] is never gathered: per tile, qexp = onehotT.T @ q_slot on the PE.
  - scores: DVE multiply (qexp*k) + Pool segment reduce (16-wide heads) +
    DVE bias add; exp on ACT; msgs = v*exp(scores) on DVE.
  - Unnormalized msgs are scatter-added with PE matmuls (msgs.T @ onehot)
    into PSUM (one bank per 4-slot block, single ascending pass), flushed
    to SBUF out_acc.
  - The per-head softmax denominator is AllReduced (32 B) across the 8 cores;
    out_acc is scaled by 1/Z, multiplied by Wo.T on the PE and written out.
"""

import math
import sys

sys.path.insert(0, "/opt/trn_rl_repo")

import numpy as np
import ml_dtypes

import concourse.bass as bass
import concourse.bacc as bacc
import concourse.mybir as mybir
import concourse.tile as tile
from concourse import bass_utils

BF16 = ml_dtypes.bfloat16
FP8 = ml_dtypes.float8_e4m3


# ---------------------------------------------------------------- config ----
class Config:
    def __init__(self, N=50000, E=800000, n_cores=8, chunk_tiles=16):
        assert N % n_cores == 0
        self.N, self.E, self.R = N, E, n_cores
        self.C, self.H, self.HD = 128, 8, 16
        self.NLOC = N // n_cores                       # nodes per core
        self.NSLOT = -(-self.NLOC // 128)              # 128-node slots
        self.NLOCP = self.NSLOT * 128                  # padded local nodes
        self.CT = chunk_tiles                          # tiles per chunk
        self.CE = chunk_tiles * 128                    # edges per chunk


FULL = Config()


# ------------------------------------------------------------- host prep ----
def _host_prep(cfg, x, edge_index, edge_weight, Wd_vec):
    """Shard + schedule. Returns (sched, per_core) where sched is shared
    compile-time metadata and per_core is a list of input dicts."""
    N, E, R = cfg.N, cfg.E, cfg.R
    row = np.asarray(edge_index[0], dtype=np.int64)
    col = np.asarray(edge_index[1], dtype=np.int64)
    ew = np.asarray(edge_weight, dtype=np.float32)
    xnp = np.asarray(x, dtype=np.float32)
    xm = xnp.mean(axis=1)                              # [N] row means
    dm_all = (xm[col] - xm[row]).astype(np.float32)    # per-edge dmean
    xTb = np.ascontiguousarray(xnp.T.astype(BF16))     # [128, N]

    # --- per-core edge lists, slot-sorted ---
    core_of = row // cfg.NLOC
    per_core_edges = []          # [r] -> dict s -> (cols, ews, dms, rel)
    counts = np.zeros((R, cfg.NSLOT), dtype=np.int64)
    for r in range(R):
        m = core_of == r
        rl = row[m] - r * cfg.NLOC
        cl, wl, dl = col[m], ew[m], dm_all[m]
        slot = rl // 128
        order = np.lexsort((rl, slot))
        rl, cl, wl, dl, slot = (a[order] for a in (rl, cl, wl, dl, slot))
        buckets = {}
        for s in range(cfg.NSLOT):
            sm = slot == s
            buckets[s] = (cl[sm], wl[sm], dl[sm], rl[sm] - s * 128)
            counts[r, s] = sm.sum()
        per_core_edges.append(buckets)

    # --- shared schedule: per-slot tile counts = max over cores ---
    ntile = np.maximum(1, -(-counts.max(axis=0) // 128))   # [NSLOT]
    T = int(ntile.sum())
    ntile[cfg.NSLOT - 1] += (-T) % cfg.CT                  # pad to chunk mult
    T = int(ntile.sum())
    tile_slot = []
    for s in range(cfg.NSLOT):
        tile_slot += [s] * int(ntile[s])
    nchunk = T // cfg.CT

    sched = dict(T=T, nchunk=nchunk, tile_slot=tile_slot)

    CT, CE, H = cfg.CT, cfg.CE, cfg.H
    MB = CT * H * 4                      # bias bytes per partition per chunk
    AUXW = MB + CE + CE + 2 * CE         # bias | oh | ohT | xgT

    per_core = []
    for r in range(R):
        cols = np.zeros(T * 128, dtype=np.int64)
        ews = np.full(T * 128, -1e9, dtype=np.float32)
        dms = np.zeros(T * 128, dtype=np.float32)
        rels = np.zeros(T * 128, dtype=np.int64)
        pos = 0
        for s in range(cfg.NSLOT):
            cc, wl, dl, rl = per_core_edges[r][s]
            n = len(cc)
            room = int(ntile[s]) * 128
            assert n <= room
            cols[pos:pos + n] = cc
            ews[pos:pos + n] = wl
            dms[pos:pos + n] = dl
            rels[pos:pos + n] = rl
            pos += room
        assert pos == T * 128

        # one-hot matrices [tile, e, w] and transpose, chunk layouts
        oh = np.zeros((T, 128, 128), dtype=FP8)
        ti = np.repeat(np.arange(T), 128)
        ei = np.tile(np.arange(128), T)
        oh[ti, ei, rels] = 1
        oh_c = (oh.reshape(nchunk, CT, 128, 128)
                  .transpose(0, 2, 1, 3).reshape(nchunk, 128, CE))
        ohT_c = (oh.transpose(0, 2, 1).reshape(nchunk, CT, 128, 128)
                   .transpose(0, 2, 1, 3).reshape(nchunk, 128, CE))
        # per-edge per-head score bias: ew + dmean*Wd  [T*128, H] f32
        bias_eh = (ews[:, None] + dms[:, None] * Wd_vec[None, :]).astype(
            np.float32)
        bias_c = (bias_eh.reshape(nchunk, CT, 128, H)
                  .transpose(0, 2, 1, 3).reshape(nchunk, 128, CT * H))
        # host-pregathered source features, transposed: [128ch, T*128]
        xgT = xTb[:, cols]                              # [128, T*128]
        xgT_c = xgT.reshape(128, nchunk, CE).transpose(1, 0, 2)

        aux = np.concatenate([
            np.ascontiguousarray(bias_c).view(np.uint8),
            np.ascontiguousarray(oh_c).view(np.uint8),
            np.ascontiguousarray(ohT_c).view(np.uint8),
            np.ascontiguousarray(xgT_c).view(np.uint8),
        ], axis=2)
        assert aux.shape == (nchunk, 128, AUXW)
        per_core.append(dict(meta=np.ascontiguousarray(aux)))
    return sched, per_core


# ---------------------------------------------------------- kernel build ----
def _build(nc, cfg, sched, has_biasA, has_biasB, has_bo):
    f32, bf16 = mybir.dt.float32, mybir.dt.bfloat16
    fp8 = mybir.dt.float8e4
    u8 = mybir.dt.uint8
    C, H, CT, CE = cfg.C, cfg.H, cfg.CT, cfg.CE
    NS = cfg.NSLOT
    T, nchunk = sched["T"], sched["nchunk"]
    NBLK = -(-NS // 4)           # 512-col psum blocks over slots
    MB = CT * H * 4
    AUXW = MB + CE + CE + 2 * CE

    # ---- I/O ----
    xTloc = nc.dram_tensor("xTloc", [128, cfg.NLOCP], bf16,
                           kind="ExternalInput").ap()
    W_A = nc.dram_tensor("W_A", [128, 2 * C], bf16, kind="ExternalInput").ap()
    biasA = nc.dram_tensor("biasA", [128, 2 * C], f32, kind="ExternalInput").ap()
    W_B = nc.dram_tensor("W_B", [128, C], bf16, kind="ExternalInput").ap()
    biasB = nc.dram_tensor("biasB", [128, C], f32, kind="ExternalInput").ap()
    WoT = nc.dram_tensor("WoT", [128, C], f32, kind="ExternalInput").ap()
    bo_r = nc.dram_tensor("bo_r", [128, C], f32, kind="ExternalInput").ap()
    ones_c = nc.dram_tensor("ones_c", [128, 1], f32, kind="ExternalInput").ap()
    Mrep = nc.dram_tensor("Mrep", [cfg.H, 128], f32, kind="ExternalInput").ap()
    meta_d = nc.dram_tensor("meta", [nchunk, 128, AUXW], u8,
                            kind="ExternalInput").ap()
    out = nc.dram_tensor("out", [cfg.NLOC, C], f32, kind="ExternalOutput").ap()

    with tile.TileContext(nc) as tc:
        with (
            tc.tile_pool(name="persist", bufs=1) as pp,
            tc.tile_pool(name="wpool", bufs=1) as wp,
            tc.tile_pool(name="io", bufs=3) as iop,
            tc.tile_pool(name="psK", bufs=2, space="PSUM") as psK,
            tc.tile_pool(name="psQ", bufs=2, space="PSUM") as psQ,
            tc.tile_pool(name="psS", bufs=2, space="PSUM") as psS,
            tc.tile_pool(name="work", bufs=3) as wk,
        ):
            # persistent SBUF
            qlocal = pp.tile([128, NS * 128], bf16, tag="qlocal")
            out_acc = pp.tile([128, NS * 128], f32, tag="out_acc")
            zacc = pp.tile([128, cfg.H], f32, tag="zacc")
            nc.vector.memset(zacc[:], 0.0)

            # weights in SBUF
            WA_sb = wp.tile([128, 2 * C], bf16, tag="WA")
            nc.sync.dma_start(WA_sb[:], W_A[:])
            WB_sb = wp.tile([128, C], bf16, tag="WB")
            nc.sync.dma_start(WB_sb[:], W_B[:])
            WoT_sb = wp.tile([128, C], f32, tag="WoT")
            nc.sync.dma_start(WoT_sb[:], WoT[:])
            bo_sb = wp.tile([128, C], f32, tag="bo")
            nc.sync.dma_start(bo_sb[:], bo_r[:])
            ones_sb = wp.tile([128, 1], f32, tag="ones")
            nc.sync.dma_start(ones_sb[:], ones_c[:])
            Mrep_sb = wp.tile([cfg.H, 128], f32, tag="Mrep")
            nc.sync.dma_start(Mrep_sb[:], Mrep[:])
            if has_biasA:
                bA_sb = wp.tile([128, 2 * C], f32, tag="bA")
                nc.sync.dma_start(bA_sb[:], biasA[:])
            if has_biasB:
                bB_sb = wp.tile([128, C], f32, tag="bB")
                nc.sync.dma_start(bB_sb[:], biasB[:])

            # ------------- P2: local q tilde -> SBUF qlocal -------------
            s = 0
            while s < NS:
                nb = min(4, NS - s)
                xt4 = iop.tile([128, 512], bf16, tag="xt2")
                nc.scalar.dma_start(xt4[:, 0:nb * 128],
                                    xTloc[:, s * 128:(s + nb) * 128])
                for j in range(nb):
                    ps = psQ.tile([128, 512], f32, tag="qexp")
                    nc.tensor.matmul(out=ps[:, 0:C],
                                     lhsT=xt4[:, j * 128:(j + 1) * 128],
                                     rhs=WB_sb[:], start=True, stop=True)
                    dstq = qlocal[:, (s + j) * 128:(s + j + 1) * 128]
                    if has_biasB:
                        nc.vector.tensor_tensor(out=dstq, in0=ps[:, 0:C],
                                                in1=bB_sb[:],
                                                op=mybir.AluOpType.add)
                    elif (s + j) % 2 == 0:
                        nc.scalar.copy(dstq, ps[:, 0:C])
                    else:
                        nc.vector.tensor_copy(dstq, ps[:, 0:C])
                s += nb

            # ---------------- P3: edge chunks ----------------
            ngroup = CT // 4
            scat_ps = None          # current scatter psum bank
            cur_blk = -1
            blk_seen = [False] * NBLK

            def flush_block(blk, ps_tile):
                lo, hi_ = blk * 4, min(blk * 4 + 4, NS)
                w = (hi_ - lo) * 128
                dst = out_acc[:, blk * 512: blk * 512 + w]
                if blk_seen[blk]:
                    nc.vector.tensor_tensor(out=dst, in0=dst,
                                            in1=ps_tile[:, 0:w],
                                            op=mybir.AluOpType.add)
                else:
                    nc.vector.tensor_copy(dst, ps_tile[:, 0:w])
                    blk_seen[blk] = True

            for c in range(nchunk):
                aux = wk.tile([128, AUXW], u8, tag="aux")
                eng = nc.sync if c % 2 == 0 else nc.scalar
                eng.dma_start(aux[:], meta_d[c, :, :])
                bias_sb = aux[:, 0:MB].bitcast(f32)            # [128, CT*H]
                oh_sb = aux[:, MB:MB + CE].bitcast(fp8)
                ohT_sb = aux[:, MB + CE:MB + 2 * CE].bitcast(fp8)
                xgT_sb = aux[:, MB + 2 * CE:].bitcast(bf16)    # [128, CE]

                exps = wk.tile([128, CT * H], bf16, tag="exps")
                for g in range(ngroup):
                    kvps = psK.tile([128, 4, 2 * C], f32, tag="kv")
                    qps = psQ.tile([128, 512], f32, tag="qexp")
                    for j in range(4):
                        t = g * 4 + j
                        nc.tensor.matmul(
                            out=kvps[:, j, :],
                            lhsT=xgT_sb[:, t * 128:(t + 1) * 128],
                            rhs=WA_sb[:], start=True, stop=True)
                    for j in range(4):
                        t = g * 4 + j
                        sl = sched["tile_slot"][c * CT + t]
                        nc.tensor.matmul(
                            out=qps[:, j * 128:(j + 1) * 128],
                            lhsT=ohT_sb[:, t * 128:(t + 1) * 128],
                            rhs=qlocal[:, sl * 128:(sl + 1) * 128],
                            start=True, stop=True)
                    if has_biasA:
                        kvsb = wk.tile([128, 4, 2 * C], f32, tag="kvsb")
                        nc.vector.tensor_tensor(
                            out=kvsb[:], in0=kvps[:],
                            in1=bA_sb[:].rearrange("p c -> p () c")
                                        .to_broadcast([128, 4, 2 * C]),
                            op=mybir.AluOpType.add)
                        kv_k = kvsb[:, :, 0:C]
                        kv_v = kvsb[:, :, C:2 * C]
                    else:
                        kv_k = kvps[:, :, 0:C]
                        kv_v = kvps[:, :, C:2 * C]
                    qbf = wk.tile([128, 512], bf16, tag="qbf")
                    nc.scalar.copy(qbf[:], qps[:])
                    prod = wk.tile([128, 512], bf16, tag="prod")
                    nc.vector.tensor_tensor(
                        out=prod[:].rearrange("p (t c) -> p t c", t=4, c=C),
                        in0=qbf[:].rearrange("p (t c) -> p t c", t=4, c=C),
                        in1=kv_k,
                        op=mybir.AluOpType.mult)
                    sc = wk.tile([128, 4 * H], f32, tag="sc")
                    nc.vector.tensor_reduce(
                        out=sc[:],
                        in_=prod[:].rearrange("p (t h d) -> p t h d",
                                              t=4, h=H, d=cfg.HD),
                        axis=mybir.AxisListType.X, op=mybir.AluOpType.add)
                    nc.vector.tensor_tensor(
                        out=sc[:], in0=sc[:],
                        in1=bias_sb[:, g * 4 * H:(g + 1) * 4 * H],
                        op=mybir.AluOpType.add)
                    exps_g = exps[:, g * 4 * H:(g + 1) * 4 * H]
                    nc.scalar.activation(exps_g, sc[:],
                                         mybir.ActivationFunctionType.Exp)
                    msgs = wk.tile([128, 512], bf16, tag="msgs")
                    nc.vector.tensor_tensor(
                        out=msgs[:].rearrange("p (t h d) -> p t h d",
                                              t=4, h=H, d=cfg.HD),
                        in0=kv_v.rearrange("p t (h d) -> p t h d",
                                           h=H, d=cfg.HD),
                        in1=exps_g.rearrange("p (t h) -> p t h ()",
                                             t=4, h=H)
                                  .to_broadcast([128, 4, H, cfg.HD]),
                        op=mybir.AluOpType.mult)
                    for j in range(4):
                        t = g * 4 + j
                        gt = c * CT + t
                        sl = sched["tile_slot"][gt]
                        blk = sl // 4
                        if blk != cur_blk:
                            if scat_ps is not None:
                                flush_block(cur_blk, scat_ps)
                            scat_ps = psS.tile([128, 512], f32, tag="scat")
                            # PSUM accumulation groups cannot survive
                            # same-bank interleaving (HW): pre-zero the bank
                            # and use start=False on every matmul instead.
                            nc.vector.memset(scat_ps[:], 0.0)
                            cur_blk = blk
                        nc.tensor.matmul(
                            out=scat_ps[:, (sl % 4) * 128:(sl % 4) * 128 + 128],
                            lhsT=msgs[:, j * C:(j + 1) * C],
                            rhs=oh_sb[:, t * 128:(t + 1) * 128],
                            start=False, stop=True, skip_group_check=True)
                # per-chunk softmax denominator accumulation (Pool)
                ztmp = wk.tile([128, H], f32, tag="ztmp")
                nc.vector.tensor_reduce(
                    out=ztmp[:],
                    in_=exps[:].rearrange("p (t h) -> p h t", t=CT, h=H),
                    axis=mybir.AxisListType.X, op=mybir.AluOpType.add)
                nc.gpsimd.tensor_tensor(out=zacc[:], in0=zacc[:],
                                        in1=ztmp[:],
                                        op=mybir.AluOpType.add)
            flush_block(cur_blk, scat_ps)

            # ---------------- P4: finale ----------------
            zsum_ps = psQ.tile([128, 512], f32, tag="qexp")
            nc.tensor.matmul(out=zsum_ps[0:1, 0:cfg.H], lhsT=ones_sb[:],
                             rhs=zacc[:], start=True, stop=True)
            zsb = wk.tile([1, cfg.H], f32, tag="zsb")
            nc.vector.tensor_copy(zsb[:], zsum_ps[0:1, 0:cfg.H])
            with tc.tile_pool(name="dram", bufs=1, space="DRAM") as dp:
                zin_d = dp.tile([1, cfg.H], f32)
                zout_d = dp.tile([1, cfg.H], f32)
                nc.sync.dma_start(zin_d[:], zsb[:])
                nc.gpsimd.collective_compute(
                    "AllReduce", mybir.AluOpType.add,
                    replica_groups=[list(range(cfg.R))],
                    ins=[zin_d.opt()], outs=[zout_d.opt()])
                zvec = wk.tile([cfg.H, 1], f32, tag="zvec")
                nc.sync.dma_start(zvec[:], zout_d[:].rearrange("a h -> h a"))
            zcol_ps = psQ.tile([128, 512], f32, tag="qexp")
            nc.tensor.matmul(out=zcol_ps[:, 0:1], lhsT=Mrep_sb[:], rhs=zvec[:],
                             start=True, stop=True)
            rz = wk.tile([128, 1], f32, tag="rz")
            nc.vector.reciprocal(rz[:], zcol_ps[:, 0:1])
            nc.vector.tensor_scalar(out=out_acc[:], in0=out_acc[:],
                                    scalar1=rz[:], scalar2=None,
                                    op0=mybir.AluOpType.mult)

            for s in range(NS):
                rows = min(128, cfg.NLOC - s * 128)
                ps = psQ.tile([128, 512], f32, tag="qexp")
                ps = ps[:, 0:C]
                nc.tensor.matmul(out=ps,
                                 lhsT=out_acc[:, s * 128:(s + 1) * 128],
                                 rhs=WoT_sb[:], start=True, stop=True)
                of = iop.tile([128, C], f32, tag="of")
                if has_bo:
                    nc.vector.tensor_tensor(out=of[:], in0=ps, in1=bo_sb[:],
                                            op=mybir.AluOpType.add)
                else:
                    nc.vector.tensor_copy(of[:], ps)
                nc.sync.dma_start(out[s * 128:s * 128 + rows, :], of[0:rows, :])
    return nc


# -------------------------------------------------------------- frontend ----
def _run(cfg, inputs, trace=False):
    x = np.asarray(inputs["x"], dtype=np.float32)
    sched, per_core = _host_prep(cfg, x, inputs["edge_index"],
                                 inputs["edge_weight"],
                                 np.asarray(inputs["Wd"],
                                            np.float32).reshape(-1))

    f32 = np.float32
    Wq = np.asarray(inputs["Wq"], f32); bq = np.asarray(inputs["bq"], f32)
    Wk = np.asarray(inputs["Wk"], f32); bk = np.asarray(inputs["bk"], f32)
    Wv = np.asarray(inputs["Wv"], f32); bv = np.asarray(inputs["bv"], f32)
    Wo = np.asarray(inputs["Wo"], f32); bo = np.asarray(inputs["bo"], f32)
    inv = 1.0 / math.sqrt(cfg.HD)

    W_A = np.concatenate([Wk.T, Wv.T], axis=1).astype(BF16)   # [128, 256]
    biasA = np.tile(np.concatenate([bk, bv])[None, :], (128, 1))
    W_B = (Wq.T * inv).astype(BF16)
    biasB = np.tile((bq * inv)[None, :], (128, 1))
    has_biasA = bool(np.any(biasA)); has_biasB = bool(np.any(biasB))
    has_bo = bool(np.any(bo))
    Mrep = np.zeros((cfg.H, 128), f32)
    for h in range(cfg.H):
        Mrep[h, h * 16:(h + 1) * 16] = 1.0

    base = dict(
        W_A=W_A, biasA=biasA.astype(f32), W_B=W_B,
        biasB=biasB.astype(f32), WoT=np.ascontiguousarray(Wo.T),
        bo_r=np.tile(bo[None, :], (128, 1)).astype(f32),
        ones_c=np.ones((128, 1), f32), Mrep=Mrep)

    in_maps = []
    for r in range(cfg.R):
        xloc = np.zeros((128, cfg.NLOCP), BF16)
        xloc[:, :cfg.NLOC] = x[r * cfg.NLOC:(r + 1) * cfg.NLOC].T.astype(BF16)
        m = dict(base)
        m["xTloc"] = xloc
        m.update(per_core[r])
        in_maps.append(m)

    nc = bacc.Bacc("TRN2", target_bir_lowering=False, debug=False,
                   num_devices=cfg.R)
    _build(nc, cfg, sched, has_biasA, has_biasB, has_bo)
    nc.compile()

    res = bass_utils.run_bass_kernel_spmd(
        nc, in_maps, core_ids=list(range(cfg.R)), trace=trace)
    outs = [res.results[r]["out"] for r in range(cfg.R)]
    full = np.concatenate(outs, axis=0).astype(np.float32)
    return full, res


def kernel(**inputs):
    out, _ = _run(FULL, inputs)
    return out


if __name__ == "__main__":
    pass


# revision 6
# speedup vs baseline: 1.7800x; 1.2326x over previous
"""Bass/Trainium2 kernel for nn_DirectionalGraphAttention (8 NeuronCores).

Math (see reference):
    q = (x@Wq.T + bq),  k = (x@Wk.T + bk),  v = (x@Wv.T + bv)      [N, C]
    scores[e,h] = q[row_e,h,:].k[col_e,h,:]/sqrt(HD) + ew_e
                  + (mean(x[col_e]) - mean(x[row_e])) * Wd[h] + bd[h]
    attn = softmax(scores, axis=0)            (global over ALL edges, per head)
    out[n,:] = (sum_{e: row_e==n} attn[e,h]*v[col_e,:]) @ Wo.T + bo

Strategy (8-way SPMD, one compiled program, per-core data differs):
  - Shard NODES into 8 contiguous ranges by destination; core r handles the
    edges whose row lands in its range (counts are ~E/8 by uniformity).
  - bd drops out (softmax over edges is invariant to per-head constants).
  - Per-edge source features are HOST-pregathered: for each 128-edge tile the
    host ships xgT[ch, e] = x[col_e, ch].T in bf16, packed together with the
    per-edge score bias (ew + dmean*Wd), the scatter one-hot and its
    transpose into ONE per-chunk DMA stream. No DRAM kv table, no SWDGE
    gather (the v1 design spent ~0.5 ms/core in gather descriptor ucode and
    ~5 ms of DMA-engine-seconds on 512 B gathered rows).
  - k|v are computed on the fly per tile: kv[e, 0:256] = xgT_tile.T @
    [Wk.T|Wv.T] — one 256-wide PE matmul per tile (FWL weight loads), output
    kept in PSUM (f32; skips v1's bf16 table rounding).
  - Destination rows are grouped into 128-node "slots" (sorted, padded to a
    shared compile-time schedule; pad edges have ew=-1e9 -> exp=0).
  - q[row] is never gathered: per tile, qexp = onehotT.T @ q_slot on the PE.
  - scores: DVE multiply (qexp*k) + Pool segment reduce (16-wide heads) +
    DVE bias add; exp on ACT; msgs = v*exp(scores) on DVE.
  - Unnormalized msgs are scatter-added with PE matmuls (msgs.T @ onehot)
    into PSUM (one bank per 4-slot block, single ascending pass), flushed
    to SBUF out_acc.
  - The per-head softmax denominator is AllReduced (32 B) across the 8 cores;
    out_acc is scaled by 1/Z, multiplied by Wo.T on the PE and written out.
"""

import math
import sys

sys.path.insert(0, "/opt/trn_rl_repo")

import numpy as np
import ml_dtypes

import concourse.bass as bass
import concourse.bacc as bacc
import concourse.mybir as mybir
import concourse.tile as tile
from concourse import bass_utils

BF16 = ml_dtypes.bfloat16
FP8 = ml_dtypes.float8_e4m3


# ---------------------------------------------------------------- config ----
class Config:
    def __init__(self, N=50000, E=800000, n_cores=8, chunk_tiles=16):
        assert N % n_cores == 0
        self.N, self.E, self.R = N, E, n_cores
        self.C, self.H, self.HD = 128, 8, 16
        self.NLOC = N // n_cores                       # nodes per core
        self.NSLOT = -(-self.NLOC // 128)              # 128-node slots
        self.NLOCP = self.NSLOT * 128                  # padded local nodes
        self.CT = chunk_tiles                          # tiles per chunk
        self.CE = chunk_tiles * 128                    # edges per chunk


FULL = Config()


# ------------------------------------------------------------- host prep ----
def _host_prep(cfg, x, edge_index, edge_weight, Wd_vec):
    """Shard + schedule. Returns (sched, per_core) where sched is shared
    compile-time metadata and per_core is a list of input dicts."""
    N, E, R = cfg.N, cfg.E, cfg.R
    row = np.asarray(edge_index[0], dtype=np.int64)
    col = np.asarray(edge_index[1], dtype=np.int64)
    ew = np.asarray(edge_weight, dtype=np.float32)
    xnp = np.asarray(x, dtype=np.float32)
    xm = xnp.mean(axis=1)                              # [N] row means
    dm_all = (xm[col] - xm[row]).astype(np.float32)    # per-edge dmean
    xTb = np.ascontiguousarray(xnp.T.astype(BF16))     # [128, N]

    # --- per-core edge lists, slot-sorted ---
    core_of = row // cfg.NLOC
    per_core_edges = []          # [r] -> dict s -> (cols, ews, dms, rel)
    counts = np.zeros((R, cfg.NSLOT), dtype=np.int64)
    for r in range(R):
        m = core_of == r
        rl = row[m] - r * cfg.NLOC
        cl, wl, dl = col[m], ew[m], dm_all[m]
        slot = rl // 128
        order = np.lexsort((rl, slot))
        rl, cl, wl, dl, slot = (a[order] for a in (rl, cl, wl, dl, slot))
        buckets = {}
        for s in range(cfg.NSLOT):
            sm = slot == s
            buckets[s] = (cl[sm], wl[sm], dl[sm], rl[sm] - s * 128)
            counts[r, s] = sm.sum()
        per_core_edges.append(buckets)

    # --- shared schedule: per-slot tile counts = max over cores ---
    ntile = np.maximum(1, -(-counts.max(axis=0) // 128))   # [NSLOT]
    T = int(ntile.sum())
    ntile[cfg.NSLOT - 1] += (-T) % cfg.CT                  # pad to chunk mult
    T = int(ntile.sum())
    tile_slot = []
    for s in range(cfg.NSLOT):
        tile_slot += [s] * int(ntile[s])
    nchunk = T // cfg.CT

    sched = dict(T=T, nchunk=nchunk, tile_slot=tile_slot)

    CT, CE, H = cfg.CT, cfg.CE, cfg.H
    MB = CT * H * 4                      # bias bytes per partition per chunk
    AUXW = MB + CE + CE + 2 * CE         # bias | oh | ohT | xgT

    per_core = []
    for r in range(R):
        cols = np.zeros(T * 128, dtype=np.int64)
        ews = np.full(T * 128, -1e9, dtype=np.float32)
        dms = np.zeros(T * 128, dtype=np.float32)
        rels = np.zeros(T * 128, dtype=np.int64)
        pos = 0
        for s in range(cfg.NSLOT):
            cc, wl, dl, rl = per_core_edges[r][s]
            n = len(cc)
            room = int(ntile[s]) * 128
            assert n <= room
            cols[pos:pos + n] = cc
            ews[pos:pos + n] = wl
            dms[pos:pos + n] = dl
            rels[pos:pos + n] = rl
            pos += room
        assert pos == T * 128

        # one-hot matrices [tile, e, w] and transpose, chunk layouts
        oh = np.zeros((T, 128, 128), dtype=FP8)
        ti = np.repeat(np.arange(T), 128)
        ei = np.tile(np.arange(128), T)
        oh[ti, ei, rels] = 1
        oh_c = (oh.reshape(nchunk, CT, 128, 128)
                  .transpose(0, 2, 1, 3).reshape(nchunk, 128, CE))
        ohT_c = (oh.transpose(0, 2, 1).reshape(nchunk, CT, 128, 128)
                   .transpose(0, 2, 1, 3).reshape(nchunk, 128, CE))
        # per-edge per-head score bias: ew + dmean*Wd  [T*128, H] f32
        bias_eh = (ews[:, None] + dms[:, None] * Wd_vec[None, :]).astype(
            np.float32)
        bias_c = (bias_eh.reshape(nchunk, CT, 128, H)
                  .transpose(0, 2, 1, 3).reshape(nchunk, 128, CT * H))
        # host-pregathered source features, transposed: [128ch, T*128]
        xgT = xTb[:, cols]                              # [128, T*128]
        xgT_c = xgT.reshape(128, nchunk, CE).transpose(1, 0, 2)

        aux = np.concatenate([
            np.ascontiguousarray(bias_c).view(np.uint8),
            np.ascontiguousarray(oh_c).view(np.uint8),
            np.ascontiguousarray(ohT_c).view(np.uint8),
            np.ascontiguousarray(xgT_c).view(np.uint8),
        ], axis=2)
        assert aux.shape == (nchunk, 128, AUXW)
        per_core.append(dict(meta=np.ascontiguousarray(aux)))
    return sched, per_core


# ---------------------------------------------------------- kernel build ----
def _build(nc, cfg, sched, has_biasA, has_biasB, has_bo):
    f32, bf16 = mybir.dt.float32, mybir.dt.bfloat16
    fp8 = mybir.dt.float8e4
    u8 = mybir.dt.uint8
    C, H, CT, CE = cfg.C, cfg.H, cfg.CT, cfg.CE
    NS = cfg.NSLOT
    T, nchunk = sched["T"], sched["nchunk"]
    NBLK = -(-NS // 4)           # 512-col psum blocks over slots
    MB = CT * H * 4
    AUXW = MB + CE + CE + 2 * CE

    # ---- I/O ----
    xTloc = nc.dram_tensor("xTloc", [128, cfg.NLOCP], bf16,
                           kind="ExternalInput").ap()
    W_A = nc.dram_tensor("W_A", [128, 2 * C], bf16, kind="ExternalInput").ap()
    biasA = nc.dram_tensor("biasA", [128, 2 * C], f32, kind="ExternalInput").ap()
    W_B = nc.dram_tensor("W_B", [128, C], bf16, kind="ExternalInput").ap()
    biasB = nc.dram_tensor("biasB", [128, C], f32, kind="ExternalInput").ap()
    WoT = nc.dram_tensor("WoT", [128, C], f32, kind="ExternalInput").ap()
    bo_r = nc.dram_tensor("bo_r", [128, C], f32, kind="ExternalInput").ap()
    ones_c = nc.dram_tensor("ones_c", [128, 1], f32, kind="ExternalInput").ap()
    Mrep = nc.dram_tensor("Mrep", [cfg.H, 128], f32, kind="ExternalInput").ap()
    meta_d = nc.dram_tensor("meta", [nchunk, 128, AUXW], u8,
                            kind="ExternalInput").ap()
    out = nc.dram_tensor("out", [cfg.NLOC, C], f32, kind="ExternalOutput").ap()

    with tile.TileContext(nc) as tc:
        with (
            tc.tile_pool(name="persist", bufs=1) as pp,
            tc.tile_pool(name="wpool", bufs=1) as wp,
            tc.tile_pool(name="io", bufs=3) as iop,
            tc.tile_pool(name="psK", bufs=2, space="PSUM") as psK,
            tc.tile_pool(name="psQ", bufs=2, space="PSUM") as psQ,
            tc.tile_pool(name="psS", bufs=2, space="PSUM") as psS,
            tc.tile_pool(name="work", bufs=3) as wk,
        ):
            # persistent SBUF
            qlocal = pp.tile([128, NS * 128], bf16, tag="qlocal")
            out_acc = pp.tile([128, NS * 128], f32, tag="out_acc")
            zacc = pp.tile([128, cfg.H], f32, tag="zacc")
            nc.vector.memset(zacc[:], 0.0)

            # weights in SBUF
            WA_sb = wp.tile([128, 2 * C], bf16, tag="WA")
            nc.sync.dma_start(WA_sb[:], W_A[:])
            WB_sb = wp.tile([128, C], bf16, tag="WB")
            nc.sync.dma_start(WB_sb[:], W_B[:])
            WoT_sb = wp.tile([128, C], f32, tag="WoT")
            nc.sync.dma_start(WoT_sb[:], WoT[:])
            bo_sb = wp.tile([128, C], f32, tag="bo")
            nc.sync.dma_start(bo_sb[:], bo_r[:])
            ones_sb = wp.tile([128, 1], f32, tag="ones")
            nc.sync.dma_start(ones_sb[:], ones_c[:])
            Mrep_sb = wp.tile([cfg.H, 128], f32, tag="Mrep")
            nc.sync.dma_start(Mrep_sb[:], Mrep[:])
            if has_biasA:
                bA_sb = wp.tile([128, 2 * C], f32, tag="bA")
                nc.sync.dma_start(bA_sb[:], biasA[:])
            if has_biasB:
                bB_sb = wp.tile([128, C], f32, tag="bB")
                nc.sync.dma_start(bB_sb[:], biasB[:])

            # ------------- P2: local q tilde -> SBUF qlocal -------------
            s = 0
            while s < NS:
                nb = min(4, NS - s)
                xt4 = iop.tile([128, 512], bf16, tag="xt2")
                nc.scalar.dma_start(xt4[:, 0:nb * 128],
                                    xTloc[:, s * 128:(s + nb) * 128])
                for j in range(nb):
                    ps = psQ.tile([128, 512], f32, tag="qexp")
                    nc.tensor.matmul(out=ps[:, 0:C],
                                     lhsT=xt4[:, j * 128:(j + 1) * 128],
                                     rhs=WB_sb[:], start=True, stop=True)
                    dstq = qlocal[:, (s + j) * 128:(s + j + 1) * 128]
                    if has_biasB:
                        nc.vector.tensor_tensor(out=dstq, in0=ps[:, 0:C],
                                                in1=bB_sb[:],
                                                op=mybir.AluOpType.add)
                    elif (s + j) % 2 == 0:
                        nc.scalar.copy(dstq, ps[:, 0:C])
                    else:
                        nc.vector.tensor_copy(dstq, ps[:, 0:C])
                s += nb

            # ---------------- P3: edge chunks ----------------
            ngroup = CT // 4
            blk_seen = [False] * NBLK
            st = dict(scat_ps=None, cur_blk=-1)

            def flush_block(blk, ps_tile):
                lo, hi_ = blk * 4, min(blk * 4 + 4, NS)
                w = (hi_ - lo) * 128
                dst = out_acc[:, blk * 512: blk * 512 + w]
                if blk_seen[blk]:
                    nc.vector.tensor_tensor(out=dst, in0=dst,
                                            in1=ps_tile[:, 0:w],
                                            op=mybir.AluOpType.add)
                else:
                    nc.vector.tensor_copy(dst, ps_tile[:, 0:w])
                    blk_seen[blk] = True

            def issue_scatter(gt, msgs_t, j, oh_ap):
                sl = sched["tile_slot"][gt]
                blk = sl // 4
                if blk != st["cur_blk"]:
                    if st["scat_ps"] is not None:
                        flush_block(st["cur_blk"], st["scat_ps"])
                    st["scat_ps"] = psS.tile([128, 512], f32, tag="scat",
                                             name="scat_ps")
                    # PSUM accumulation groups cannot survive same-bank
                    # interleaving (HW): pre-zero the bank and use
                    # start=False on every matmul instead.
                    nc.vector.memset(st["scat_ps"][:], 0.0)
                    st["cur_blk"] = blk
                nc.tensor.matmul(
                    out=st["scat_ps"][:, (sl % 4) * 128:(sl % 4) * 128 + 128],
                    lhsT=msgs_t[:, j * C:(j + 1) * C],
                    rhs=oh_ap,
                    start=False, stop=True, skip_group_check=True)

            # scatter matmuls are issued PD tiles behind their msgs so the
            # in-order PE queue never blocks on the DVE/ACT score chain
            PD = 8
            pending = []
            PF = 2                   # aux DMA prefetch depth (chunks)
            aux_tiles = {}

            def load_aux(ci):
                a = wk.tile([128, AUXW], u8, tag="aux", bufs=4)
                eng = nc.sync if ci % 2 == 0 else nc.scalar
                eng.dma_start(a[:], meta_d[ci, :, :])
                aux_tiles[ci] = a

            for ci in range(min(PF + 1, nchunk)):
                load_aux(ci)

            for c in range(nchunk):
                if c + PF + 1 < nchunk:
                    load_aux(c + PF + 1)
                aux = aux_tiles.pop(c)
                bias_sb = aux[:, 0:MB].bitcast(f32)            # [128, CT*H]
                oh_sb = aux[:, MB:MB + CE].bitcast(fp8)
                ohT_sb = aux[:, MB + CE:MB + 2 * CE].bitcast(fp8)
                xgT_sb = aux[:, MB + 2 * CE:].bitcast(bf16)    # [128, CE]

                exps = wk.tile([128, CT * H], bf16, tag="exps")
                for g in range(ngroup):
                    kvps = psK.tile([128, 4, 2 * C], f32, tag="kv")
                    qps = psQ.tile([128, 512], f32, tag="qexp")
                    for j in range(4):
                        t = g * 4 + j
                        nc.tensor.matmul(
                            out=kvps[:, j, :],
                            lhsT=xgT_sb[:, t * 128:(t + 1) * 128],
                            rhs=WA_sb[:], start=True, stop=True)
                    for j in range(4):
                        t = g * 4 + j
                        sl = sched["tile_slot"][c * CT + t]
                        nc.tensor.matmul(
                            out=qps[:, j * 128:(j + 1) * 128],
                            lhsT=ohT_sb[:, t * 128:(t + 1) * 128],
                            rhs=qlocal[:, sl * 128:(sl + 1) * 128],
                            start=True, stop=True)
                    while len(pending) > PD - 4:
                        issue_scatter(*pending.pop(0))
                    # kv -> SBUF bf16 in one hop (frees the PSUM bank fast;
                    # also enables 16-bit-rate DVE for msgs)
                    kvsb = wk.tile([128, 4, 2 * C], bf16, tag="kvsb", bufs=4)
                    if has_biasA:
                        nc.vector.tensor_tensor(
                            out=kvsb[:], in0=kvps[:],
                            in1=bA_sb[:].rearrange("p c -> p () c")
                                        .to_broadcast([128, 4, 2 * C]),
                            op=mybir.AluOpType.add)
                    else:
                        nc.scalar.copy(kvsb[:], kvps[:])
                    prod = wk.tile([128, 512], bf16, tag="prod")
                    nc.vector.tensor_tensor(
                        out=prod[:].rearrange("p (t c) -> p t c", t=4, c=C),
                        in0=qps[:].rearrange("p (t c) -> p t c", t=4, c=C),
                        in1=kvsb[:, :, 0:C],
                        op=mybir.AluOpType.mult)
                    sc = wk.tile([128, 4 * H], f32, tag="sc")
                    nc.vector.tensor_reduce(
                        out=sc[:],
                        in_=prod[:].rearrange("p (t h d) -> p t h d",
                                              t=4, h=H, d=cfg.HD),
                        axis=mybir.AxisListType.X, op=mybir.AluOpType.add)
                    nc.vector.tensor_tensor(
                        out=sc[:], in0=sc[:],
                        in1=bias_sb[:, g * 4 * H:(g + 1) * 4 * H],
                        op=mybir.AluOpType.add)
                    exps_g = exps[:, g * 4 * H:(g + 1) * 4 * H]
                    nc.scalar.activation(exps_g, sc[:],
                                         mybir.ActivationFunctionType.Exp)
                    msgs = wk.tile([128, 512], bf16, tag="msgs", bufs=4)
                    nc.vector.tensor_tensor(
                        out=msgs[:].rearrange("p (t h d) -> p t h d",
                                              t=4, h=H, d=cfg.HD),
                        in0=kvsb[:, :, C:2 * C].rearrange(
                            "p t (h d) -> p t h d", h=H, d=cfg.HD),
                        in1=exps_g.rearrange("p (t h) -> p t h ()",
                                             t=4, h=H)
                                  .to_broadcast([128, 4, H, cfg.HD]),
                        op=mybir.AluOpType.mult)
                    for j in range(4):
                        t = g * 4 + j
                        pending.append((c * CT + t, msgs, j,
                                        oh_sb[:, t * 128:(t + 1) * 128]))
                # per-chunk softmax denominator accumulation
                ztmp = wk.tile([128, H], f32, tag="ztmp")
                nc.vector.tensor_reduce(
                    out=ztmp[:],
                    in_=exps[:].rearrange("p (t h) -> p h t", t=CT, h=H),
                    axis=mybir.AxisListType.X, op=mybir.AluOpType.add)
                nc.gpsimd.tensor_tensor(out=zacc[:], in0=zacc[:],
                                        in1=ztmp[:],
                                        op=mybir.AluOpType.add)
            while pending:
                issue_scatter(*pending.pop(0))
            flush_block(st["cur_blk"], st["scat_ps"])

            # ---------------- P4: finale ----------------
            zsum_ps = psQ.tile([128, 512], f32, tag="qexp")
            nc.tensor.matmul(out=zsum_ps[0:1, 0:cfg.H], lhsT=ones_sb[:],
                             rhs=zacc[:], start=True, stop=True)
            zsb = wk.tile([1, cfg.H], f32, tag="zsb")
            nc.vector.tensor_copy(zsb[:], zsum_ps[0:1, 0:cfg.H])
            with tc.tile_pool(name="dram", bufs=1, space="DRAM") as dp:
                zin_d = dp.tile([1, cfg.H], f32)
                zout_d = dp.tile([1, cfg.H], f32)
                nc.sync.dma_start(zin_d[:], zsb[:])
                nc.gpsimd.collective_compute(
                    "AllReduce", mybir.AluOpType.add,
                    replica_groups=[list(range(cfg.R))],
                    ins=[zin_d.opt()], outs=[zout_d.opt()])
                zvec = wk.tile([cfg.H, 1], f32, tag="zvec")
                nc.sync.dma_start(zvec[:], zout_d[:].rearrange("a h -> h a"))
            zcol_ps = psQ.tile([128, 512], f32, tag="qexp")
            nc.tensor.matmul(out=zcol_ps[:, 0:1], lhsT=Mrep_sb[:], rhs=zvec[:],
                             start=True, stop=True)
            rz = wk.tile([128, 1], f32, tag="rz")
            nc.vector.reciprocal(rz[:], zcol_ps[:, 0:1])
            nc.vector.tensor_scalar(out=out_acc[:], in0=out_acc[:],
                                    scalar1=rz[:], scalar2=None,
                                    op0=mybir.AluOpType.mult)

            for s in range(NS):
                rows = min(128, cfg.NLOC - s * 128)
                ps = psQ.tile([128, 512], f32, tag="qexp")
                ps = ps[:, 0:C]
                nc.tensor.matmul(out=ps,
                                 lhsT=out_acc[:, s * 128:(s + 1) * 128],
                                 rhs=WoT_sb[:], start=True, stop=True)
                of = iop.tile([128, C], f32, tag="of")
                if has_bo:
                    nc.vector.tensor_tensor(out=of[:], in0=ps, in1=bo_sb[:],
                                            op=mybir.AluOpType.add)
                else:
                    nc.vector.tensor_copy(of[:], ps)
                nc.sync.dma_start(out[s * 128:s * 128 + rows, :], of[0:rows, :])
    return nc


# -------------------------------------------------------------- frontend ----
def _run(cfg, inputs, trace=False):
    x = np.asarray(inputs["x"], dtype=np.float32)
    sched, per_core = _host_prep(cfg, x, inputs["edge_index"],
                                 inputs["edge_weight"],
                                 np.asarray(inputs["Wd"],
                                            np.float32).reshape(-1))

    f32 = np.float32
    Wq = np.asarray(inputs["Wq"], f32); bq = np.asarray(inputs["bq"], f32)
    Wk = np.asarray(inputs["Wk"], f32); bk = np.asarray(inputs["bk"], f32)
    Wv = np.asarray(inputs["Wv"], f32); bv = np.asarray(inputs["bv"], f32)
    Wo = np.asarray(inputs["Wo"], f32); bo = np.asarray(inputs["bo"], f32)
    inv = 1.0 / math.sqrt(cfg.HD)

    W_A = np.concatenate([Wk.T, Wv.T], axis=1).astype(BF16)   # [128, 256]
    biasA = np.tile(np.concatenate([bk, bv])[None, :], (128, 1))
    W_B = (Wq.T * inv).astype(BF16)
    biasB = np.tile((bq * inv)[None, :], (128, 1))
    has_biasA = bool(np.any(biasA)); has_biasB = bool(np.any(biasB))
    has_bo = bool(np.any(bo))
    Mrep = np.zeros((cfg.H, 128), f32)
    for h in range(cfg.H):
        Mrep[h, h * 16:(h + 1) * 16] = 1.0

    base = dict(
        W_A=W_A, biasA=biasA.astype(f32), W_B=W_B,
        biasB=biasB.astype(f32), WoT=np.ascontiguousarray(Wo.T),
        bo_r=np.tile(bo[None, :], (128, 1)).astype(f32),
        ones_c=np.ones((128, 1), f32), Mrep=Mrep)

    in_maps = []
    for r in range(cfg.R):
        xloc = np.zeros((128, cfg.NLOCP), BF16)
        xloc[:, :cfg.NLOC] = x[r * cfg.NLOC:(r + 1) * cfg.NLOC].T.astype(BF16)
        m = dict(base)
        m["xTloc"] = xloc
        m.update(per_core[r])
        in_maps.append(m)

    nc = bacc.Bacc("TRN2", target_bir_lowering=False, debug=False,
                   num_devices=cfg.R)
    _build(nc, cfg, sched, has_biasA, has_biasB, has_bo)
    nc.compile()

    res = bass_utils.run_bass_kernel_spmd(
        nc, in_maps, core_ids=list(range(cfg.R)), trace=trace)
    outs = [res.results[r]["out"] for r in range(cfg.R)]
    full = np.concatenate(outs, axis=0).astype(np.float32)
    return full, res


def kernel(**inputs):
    out, _ = _run(FULL, inputs)
    return out


if __name__ == "__main__":
    pass


# revision 7
# speedup vs baseline: 1.7988x; 1.0106x over previous
"""Bass/Trainium2 kernel for nn_DirectionalGraphAttention (8 NeuronCores).

Math (see reference):
    q = (x@Wq.T + bq),  k = (x@Wk.T + bk),  v = (x@Wv.T + bv)      [N, C]
    scores[e,h] = q[row_e,h,:].k[col_e,h,:]/sqrt(HD) + ew_e
                  + (mean(x[col_e]) - mean(x[row_e])) * Wd[h] + bd[h]
    attn = softmax(scores, axis=0)            (global over ALL edges, per head)
    out[n,:] = (sum_{e: row_e==n} attn[e,h]*v[col_e,:]) @ Wo.T + bo

Strategy (8-way SPMD, one compiled program, per-core data differs):
  - Shard NODES into 8 contiguous ranges by destination; core r handles the
    edges whose row lands in its range (counts are ~E/8 by uniformity).
  - bd drops out (softmax over edges is invariant to per-head constants).
  - Per-edge source features are HOST-pregathered: for each 128-edge tile the
    host ships xgT[ch, e] = x[col_e, ch].T in bf16, packed together with the
    per-edge score bias (ew + dmean*Wd), the scatter one-hot and its
    transpose into ONE per-chunk DMA stream. No DRAM kv table, no SWDGE
    gather (the v1 design spent ~0.5 ms/core in gather descriptor ucode and
    ~5 ms of DMA-engine-seconds on 512 B gathered rows).
  - k|v are computed on the fly per tile: kv[e, 0:256] = xgT_tile.T @
    [Wk.T|Wv.T] — one 256-wide PE matmul per tile (FWL weight loads), output
    kept in PSUM (f32; skips v1's bf16 table rounding).
  - Destination rows are grouped into 128-node "slots" (sorted, padded to a
    shared compile-time schedule; pad edges have ew=-1e9 -> exp=0).
  - q[row] is never gathered: per tile, qexp = onehotT.T @ q_slot on the PE.
  - scores: DVE multiply (qexp*k) + Pool segment reduce (16-wide heads) +
    DVE bias add; exp on ACT; msgs = v*exp(scores) on DVE.
  - Unnormalized msgs are scatter-added with PE matmuls (msgs.T @ onehot)
    into PSUM (one bank per 4-slot block, single ascending pass), flushed
    to SBUF out_acc.
  - The per-head softmax denominator is AllReduced (32 B) across the 8 cores;
    out_acc is scaled by 1/Z, multiplied by Wo.T on the PE and written out.
"""

import math
import sys

sys.path.insert(0, "/opt/trn_rl_repo")

import numpy as np
import ml_dtypes

import concourse.bass as bass
import concourse.bacc as bacc
import concourse.mybir as mybir
import concourse.tile as tile
from concourse import bass_utils

BF16 = ml_dtypes.bfloat16
FP8 = ml_dtypes.float8_e4m3


# ---------------------------------------------------------------- config ----
class Config:
    def __init__(self, N=50000, E=800000, n_cores=8, chunk_tiles=16):
        assert N % n_cores == 0
        self.N, self.E, self.R = N, E, n_cores
        self.C, self.H, self.HD = 128, 8, 16
        self.NLOC = N // n_cores                       # nodes per core
        self.NSLOT = -(-self.NLOC // 128)              # 128-node slots
        self.NLOCP = self.NSLOT * 128                  # padded local nodes
        self.CT = chunk_tiles                          # tiles per chunk
        self.CE = chunk_tiles * 128                    # edges per chunk


FULL = Config()


# ------------------------------------------------------------- host prep ----
def _host_prep(cfg, x, edge_index, edge_weight, Wd_vec):
    """Shard + schedule. Returns (sched, per_core) where sched is shared
    compile-time metadata and per_core is a list of input dicts."""
    N, E, R = cfg.N, cfg.E, cfg.R
    row = np.asarray(edge_index[0], dtype=np.int64)
    col = np.asarray(edge_index[1], dtype=np.int64)
    ew = np.asarray(edge_weight, dtype=np.float32)
    xnp = np.asarray(x, dtype=np.float32)
    xm = xnp.mean(axis=1)                              # [N] row means
    dm_all = (xm[col] - xm[row]).astype(np.float32)    # per-edge dmean
    xTb = np.ascontiguousarray(xnp.T.astype(BF16))     # [128, N]

    # --- per-core edge lists, slot-sorted ---
    core_of = row // cfg.NLOC
    per_core_edges = []          # [r] -> dict s -> (cols, ews, dms, rel)
    counts = np.zeros((R, cfg.NSLOT), dtype=np.int64)
    for r in range(R):
        m = core_of == r
        rl = row[m] - r * cfg.NLOC
        cl, wl, dl = col[m], ew[m], dm_all[m]
        slot = rl // 128
        order = np.lexsort((rl, slot))
        rl, cl, wl, dl, slot = (a[order] for a in (rl, cl, wl, dl, slot))
        buckets = {}
        for s in range(cfg.NSLOT):
            sm = slot == s
            buckets[s] = (cl[sm], wl[sm], dl[sm], rl[sm] - s * 128)
            counts[r, s] = sm.sum()
        per_core_edges.append(buckets)

    # --- shared schedule: per-slot tile counts = max over cores ---
    ntile = np.maximum(1, -(-counts.max(axis=0) // 128))   # [NSLOT]
    T = int(ntile.sum())
    ntile[cfg.NSLOT - 1] += (-T) % cfg.CT                  # pad to chunk mult
    T = int(ntile.sum())
    tile_slot = []
    for s in range(cfg.NSLOT):
        tile_slot += [s] * int(ntile[s])
    nchunk = T // cfg.CT

    sched = dict(T=T, nchunk=nchunk, tile_slot=tile_slot)

    CT, CE, H = cfg.CT, cfg.CE, cfg.H
    MB = CT * H * 4                      # bias bytes per partition per chunk
    AUXW = MB + CE + CE + 2 * CE         # bias | oh | ohT | xgT

    per_core = []
    for r in range(R):
        cols = np.zeros(T * 128, dtype=np.int64)
        ews = np.full(T * 128, -1e9, dtype=np.float32)
        dms = np.zeros(T * 128, dtype=np.float32)
        rels = np.zeros(T * 128, dtype=np.int64)
        pos = 0
        for s in range(cfg.NSLOT):
            cc, wl, dl, rl = per_core_edges[r][s]
            n = len(cc)
            room = int(ntile[s]) * 128
            assert n <= room
            cols[pos:pos + n] = cc
            ews[pos:pos + n] = wl
            dms[pos:pos + n] = dl
            rels[pos:pos + n] = rl
            pos += room
        assert pos == T * 128

        # one-hot matrices [tile, e, w] and transpose, chunk layouts
        oh = np.zeros((T, 128, 128), dtype=FP8)
        ti = np.repeat(np.arange(T), 128)
        ei = np.tile(np.arange(128), T)
        oh[ti, ei, rels] = 1
        oh_c = (oh.reshape(nchunk, CT, 128, 128)
                  .transpose(0, 2, 1, 3).reshape(nchunk, 128, CE))
        ohT_c = (oh.transpose(0, 2, 1).reshape(nchunk, CT, 128, 128)
                   .transpose(0, 2, 1, 3).reshape(nchunk, 128, CE))
        # per-edge per-head score bias: ew + dmean*Wd  [T*128, H] f32
        bias_eh = (ews[:, None] + dms[:, None] * Wd_vec[None, :]).astype(
            np.float32)
        bias_c = (bias_eh.reshape(nchunk, CT, 128, H)
                  .transpose(0, 2, 1, 3).reshape(nchunk, 128, CT * H))
        # host-pregathered source features, transposed: [128ch, T*128]
        xgT = xTb[:, cols]                              # [128, T*128]
        xgT_c = xgT.reshape(128, nchunk, CE).transpose(1, 0, 2)

        aux = np.concatenate([
            np.ascontiguousarray(bias_c).view(np.uint8),
            np.ascontiguousarray(oh_c).view(np.uint8),
            np.ascontiguousarray(ohT_c).view(np.uint8),
            np.ascontiguousarray(xgT_c).view(np.uint8),
        ], axis=2)
        assert aux.shape == (nchunk, 128, AUXW)
        per_core.append(dict(meta=np.ascontiguousarray(aux)))
    return sched, per_core


# ---------------------------------------------------------- kernel build ----
def _build(nc, cfg, sched, has_biasA, has_biasB, has_bo):
    f32, bf16 = mybir.dt.float32, mybir.dt.bfloat16
    fp8 = mybir.dt.float8e4
    u8 = mybir.dt.uint8
    C, H, CT, CE = cfg.C, cfg.H, cfg.CT, cfg.CE
    NS = cfg.NSLOT
    T, nchunk = sched["T"], sched["nchunk"]
    NBLK = -(-NS // 4)           # 512-col psum blocks over slots
    MB = CT * H * 4
    AUXW = MB + CE + CE + 2 * CE

    # ---- I/O ----
    xTloc = nc.dram_tensor("xTloc", [128, cfg.NLOCP], bf16,
                           kind="ExternalInput").ap()
    W_A = nc.dram_tensor("W_A", [128, 2 * C], bf16, kind="ExternalInput").ap()
    biasA = nc.dram_tensor("biasA", [128, 2 * C], f32, kind="ExternalInput").ap()
    W_B = nc.dram_tensor("W_B", [128, C], bf16, kind="ExternalInput").ap()
    biasB = nc.dram_tensor("biasB", [128, C], f32, kind="ExternalInput").ap()
    WoT = nc.dram_tensor("WoT", [128, C], f32, kind="ExternalInput").ap()
    bo_r = nc.dram_tensor("bo_r", [128, C], f32, kind="ExternalInput").ap()
    ones_c = nc.dram_tensor("ones_c", [128, 1], f32, kind="ExternalInput").ap()
    Mrep = nc.dram_tensor("Mrep", [cfg.H, 128], f32, kind="ExternalInput").ap()
    meta_d = nc.dram_tensor("meta", [nchunk, 128, AUXW], u8,
                            kind="ExternalInput").ap()
    out = nc.dram_tensor("out", [cfg.NLOC, C], f32, kind="ExternalOutput").ap()

    with tile.TileContext(nc) as tc:
        with (
            tc.tile_pool(name="persist", bufs=1) as pp,
            tc.tile_pool(name="wpool", bufs=1) as wp,
            tc.tile_pool(name="io", bufs=3) as iop,
            tc.tile_pool(name="psK", bufs=2, space="PSUM") as psK,
            tc.tile_pool(name="psQ", bufs=2, space="PSUM") as psQ,
            tc.tile_pool(name="psS", bufs=2, space="PSUM") as psS,
            tc.tile_pool(name="work", bufs=3) as wk,
        ):
            # persistent SBUF
            qlocal = pp.tile([128, NS * 128], bf16, tag="qlocal")
            out_acc = pp.tile([128, NS * 128], f32, tag="out_acc")
            zacc = pp.tile([128, cfg.H], f32, tag="zacc")
            nc.vector.memset(zacc[:], 0.0)

            # weights in SBUF
            WA_sb = wp.tile([128, 2 * C], bf16, tag="WA")
            nc.sync.dma_start(WA_sb[:], W_A[:])
            WB_sb = wp.tile([128, C], bf16, tag="WB")
            nc.sync.dma_start(WB_sb[:], W_B[:])
            WoT_sb = wp.tile([128, C], f32, tag="WoT")
            nc.sync.dma_start(WoT_sb[:], WoT[:])
            bo_sb = wp.tile([128, C], f32, tag="bo")
            nc.sync.dma_start(bo_sb[:], bo_r[:])
            ones_sb = wp.tile([128, 1], f32, tag="ones")
            nc.sync.dma_start(ones_sb[:], ones_c[:])
            Mrep_sb = wp.tile([cfg.H, 128], f32, tag="Mrep")
            nc.sync.dma_start(Mrep_sb[:], Mrep[:])
            if has_biasA:
                bA_sb = wp.tile([128, 2 * C], f32, tag="bA")
                nc.sync.dma_start(bA_sb[:], biasA[:])
            if has_biasB:
                bB_sb = wp.tile([128, C], f32, tag="bB")
                nc.sync.dma_start(bB_sb[:], biasB[:])

            # ------------- P2: local q tilde -> SBUF qlocal -------------
            s = 0
            while s < NS:
                nb = min(4, NS - s)
                xt4 = iop.tile([128, 512], bf16, tag="xt2")
                nc.scalar.dma_start(xt4[:, 0:nb * 128],
                                    xTloc[:, s * 128:(s + nb) * 128])
                for j in range(nb):
                    ps = psQ.tile([128, 512], f32, tag="qexp")
                    nc.tensor.matmul(out=ps[:, 0:C],
                                     lhsT=xt4[:, j * 128:(j + 1) * 128],
                                     rhs=WB_sb[:], start=True, stop=True)
                    dstq = qlocal[:, (s + j) * 128:(s + j + 1) * 128]
                    if has_biasB:
                        nc.vector.tensor_tensor(out=dstq, in0=ps[:, 0:C],
                                                in1=bB_sb[:],
                                                op=mybir.AluOpType.add)
                    elif (s + j) % 2 == 0:
                        nc.scalar.copy(dstq, ps[:, 0:C])
                    else:
                        nc.vector.tensor_copy(dstq, ps[:, 0:C])
                s += nb

            # ---------------- P3: edge chunks ----------------
            ngroup = CT // 4
            blk_seen = [False] * NBLK
            st = dict(scat_ps=None, cur_blk=-1)

            def flush_block(blk, ps_tile):
                lo, hi_ = blk * 4, min(blk * 4 + 4, NS)
                w = (hi_ - lo) * 128
                dst = out_acc[:, blk * 512: blk * 512 + w]
                if blk_seen[blk]:
                    nc.vector.tensor_tensor(out=dst, in0=dst,
                                            in1=ps_tile[:, 0:w],
                                            op=mybir.AluOpType.add)
                else:
                    nc.vector.tensor_copy(dst, ps_tile[:, 0:w])
                    blk_seen[blk] = True

            def issue_scatter(gt, msgs_t, j, oh_ap):
                sl = sched["tile_slot"][gt]
                blk = sl // 4
                if blk != st["cur_blk"]:
                    if st["scat_ps"] is not None:
                        flush_block(st["cur_blk"], st["scat_ps"])
                    st["scat_ps"] = psS.tile([128, 512], f32, tag="scat",
                                             name="scat_ps")
                    # PSUM accumulation groups cannot survive same-bank
                    # interleaving (HW): pre-zero the bank and use
                    # start=False on every matmul instead.
                    nc.vector.memset(st["scat_ps"][:], 0.0)
                    st["cur_blk"] = blk
                nc.tensor.matmul(
                    out=st["scat_ps"][:, (sl % 4) * 128:(sl % 4) * 128 + 128],
                    lhsT=msgs_t[:, j * C:(j + 1) * C],
                    rhs=oh_ap,
                    start=False, stop=True, skip_group_check=True)

            # scatter matmuls are issued PD tiles behind their msgs so the
            # in-order PE queue never blocks on the DVE/ACT score chain
            PD = 8
            pending = []
            PF = 2                   # aux DMA prefetch depth (chunks)
            aux_tiles = {}

            def load_aux(ci):
                a = wk.tile([128, AUXW], u8, tag="aux", bufs=4)
                eng = nc.sync if ci % 2 == 0 else nc.scalar
                eng.dma_start(a[:], meta_d[ci, :, :])
                aux_tiles[ci] = a

            for ci in range(min(PF + 1, nchunk)):
                load_aux(ci)

            for c in range(nchunk):
                if c + PF + 1 < nchunk:
                    load_aux(c + PF + 1)
                aux = aux_tiles.pop(c)
                bias_sb = aux[:, 0:MB].bitcast(f32)            # [128, CT*H]
                oh_sb = aux[:, MB:MB + CE].bitcast(fp8)
                ohT_sb = aux[:, MB + CE:MB + 2 * CE].bitcast(fp8)
                xgT_sb = aux[:, MB + 2 * CE:].bitcast(bf16)    # [128, CE]

                exps = wk.tile([128, CT * H], bf16, tag="exps")
                for g in range(ngroup):
                    kvps = psK.tile([128, 4, 2 * C], f32, tag="kv")
                    qps = psQ.tile([128, 512], f32, tag="qexp")
                    for j in range(4):
                        t = g * 4 + j
                        nc.tensor.matmul(
                            out=kvps[:, j, :],
                            lhsT=xgT_sb[:, t * 128:(t + 1) * 128],
                            rhs=WA_sb[:], start=True, stop=True)
                    for j in range(4):
                        t = g * 4 + j
                        sl = sched["tile_slot"][c * CT + t]
                        nc.tensor.matmul(
                            out=qps[:, j * 128:(j + 1) * 128],
                            lhsT=ohT_sb[:, t * 128:(t + 1) * 128],
                            rhs=qlocal[:, sl * 128:(sl + 1) * 128],
                            start=True, stop=True)
                    while len(pending) > PD - 4:
                        issue_scatter(*pending.pop(0))
                    # kv -> SBUF bf16 in one hop (frees the PSUM bank fast;
                    # also enables 16-bit-rate DVE for msgs)
                    kvsb = wk.tile([128, 4, 2 * C], bf16, tag="kvsb", bufs=4)
                    if has_biasA:
                        nc.vector.tensor_tensor(
                            out=kvsb[:], in0=kvps[:],
                            in1=bA_sb[:].rearrange("p c -> p () c")
                                        .to_broadcast([128, 4, 2 * C]),
                            op=mybir.AluOpType.add)
                    else:
                        nc.scalar.copy(kvsb[:], kvps[:])
                    prod = wk.tile([128, 512], bf16, tag="prod")
                    nc.vector.tensor_tensor(
                        out=prod[:].rearrange("p (t c) -> p t c", t=4, c=C),
                        in0=qps[:].rearrange("p (t c) -> p t c", t=4, c=C),
                        in1=kvsb[:, :, 0:C],
                        op=mybir.AluOpType.mult)
                    sc = wk.tile([128, 4 * H], f32, tag="sc")
                    nc.vector.tensor_reduce(
                        out=sc[:],
                        in_=prod[:].rearrange("p (t h d) -> p t h d",
                                              t=4, h=H, d=cfg.HD),
                        axis=mybir.AxisListType.X, op=mybir.AluOpType.add)
                    nc.gpsimd.tensor_tensor(
                        out=sc[:], in0=sc[:],
                        in1=bias_sb[:, g * 4 * H:(g + 1) * 4 * H],
                        op=mybir.AluOpType.add)
                    exps_g = exps[:, g * 4 * H:(g + 1) * 4 * H]
                    nc.scalar.activation(exps_g, sc[:],
                                         mybir.ActivationFunctionType.Exp)
                    msgs = wk.tile([128, 512], bf16, tag="msgs", bufs=4)
                    nc.gpsimd.tensor_tensor(
                        out=msgs[:].rearrange("p (t h d) -> p t h d",
                                              t=4, h=H, d=cfg.HD),
                        in0=kvsb[:, :, C:2 * C].rearrange(
                            "p t (h d) -> p t h d", h=H, d=cfg.HD),
                        in1=exps_g.rearrange("p (t h) -> p t h ()",
                                             t=4, h=H)
                                  .to_broadcast([128, 4, H, cfg.HD]),
                        op=mybir.AluOpType.mult)
                    for j in range(4):
                        t = g * 4 + j
                        pending.append((c * CT + t, msgs, j,
                                        oh_sb[:, t * 128:(t + 1) * 128]))
                # per-chunk softmax denominator accumulation
                ztmp = wk.tile([128, H], f32, tag="ztmp")
                nc.vector.tensor_reduce(
                    out=ztmp[:],
                    in_=exps[:].rearrange("p (t h) -> p h t", t=CT, h=H),
                    axis=mybir.AxisListType.X, op=mybir.AluOpType.add)
                nc.gpsimd.tensor_tensor(out=zacc[:], in0=zacc[:],
                                        in1=ztmp[:],
                                        op=mybir.AluOpType.add)
            while pending:
                issue_scatter(*pending.pop(0))
            flush_block(st["cur_blk"], st["scat_ps"])

            # ---------------- P4: finale ----------------
            zsum_ps = psQ.tile([128, 512], f32, tag="qexp")
            nc.tensor.matmul(out=zsum_ps[0:1, 0:cfg.H], lhsT=ones_sb[:],
                             rhs=zacc[:], start=True, stop=True)
            zsb = wk.tile([1, cfg.H], f32, tag="zsb")
            nc.vector.tensor_copy(zsb[:], zsum_ps[0:1, 0:cfg.H])
            with tc.tile_pool(name="dram", bufs=1, space="DRAM") as dp:
                zin_d = dp.tile([1, cfg.H], f32)
                zout_d = dp.tile([1, cfg.H], f32)
                nc.sync.dma_start(zin_d[:], zsb[:])
                nc.gpsimd.collective_compute(
                    "AllReduce", mybir.AluOpType.add,
                    replica_groups=[list(range(cfg.R))],
                    ins=[zin_d.opt()], outs=[zout_d.opt()])
                zvec = wk.tile([cfg.H, 1], f32, tag="zvec")
                nc.sync.dma_start(zvec[:], zout_d[:].rearrange("a h -> h a"))
            zcol_ps = psQ.tile([128, 512], f32, tag="qexp")
            nc.tensor.matmul(out=zcol_ps[:, 0:1], lhsT=Mrep_sb[:], rhs=zvec[:],
                             start=True, stop=True)
            rz = wk.tile([128, 1], f32, tag="rz")
            nc.vector.reciprocal(rz[:], zcol_ps[:, 0:1])
            nc.vector.tensor_scalar(out=out_acc[:], in0=out_acc[:],
                                    scalar1=rz[:], scalar2=None,
                                    op0=mybir.AluOpType.mult)

            for s in range(NS):
                rows = min(128, cfg.NLOC - s * 128)
                ps = psQ.tile([128, 512], f32, tag="qexp")
                ps = ps[:, 0:C]
                nc.tensor.matmul(out=ps,
                                 lhsT=out_acc[:, s * 128:(s + 1) * 128],
                                 rhs=WoT_sb[:], start=True, stop=True)
                of = iop.tile([128, C], f32, tag="of")
                if has_bo:
                    nc.vector.tensor_tensor(out=of[:], in0=ps, in1=bo_sb[:],
                                            op=mybir.AluOpType.add)
                else:
                    nc.vector.tensor_copy(of[:], ps)
                nc.sync.dma_start(out[s * 128:s * 128 + rows, :], of[0:rows, :])
    return nc


# -------------------------------------------------------------- frontend ----
def _run(cfg, inputs, trace=False):
    x = np.asarray(inputs["x"], dtype=np.float32)
    sched, per_core = _host_prep(cfg, x, inputs["edge_index"],
                                 inputs["edge_weight"],
                                 np.asarray(inputs["Wd"],
                                            np.float32).reshape(-1))

    f32 = np.float32
    Wq = np.asarray(inputs["Wq"], f32); bq = np.asarray(inputs["bq"], f32)
    Wk = np.asarray(inputs["Wk"], f32); bk = np.asarray(inputs["bk"], f32)
    Wv = np.asarray(inputs["Wv"], f32); bv = np.asarray(inputs["bv"], f32)
    Wo = np.asarray(inputs["Wo"], f32); bo = np.asarray(inputs["bo"], f32)
    inv = 1.0 / math.sqrt(cfg.HD)

    W_A = np.concatenate([Wk.T, Wv.T], axis=1).astype(BF16)   # [128, 256]
    biasA = np.tile(np.concatenate([bk, bv])[None, :], (128, 1))
    W_B = (Wq.T * inv).astype(BF16)
    biasB = np.tile((bq * inv)[None, :], (128, 1))
    has_biasA = bool(np.any(biasA)); has_biasB = bool(np.any(biasB))
    has_bo = bool(np.any(bo))
    Mrep = np.zeros((cfg.H, 128), f32)
    for h in range(cfg.H):
        Mrep[h, h * 16:(h + 1) * 16] = 1.0

    base = dict(
        W_A=W_A, biasA=biasA.astype(f32), W_B=W_B,
        biasB=biasB.astype(f32), WoT=np.ascontiguousarray(Wo.T),
        bo_r=np.tile(bo[None, :], (128, 1)).astype(f32),
        ones_c=np.ones((128, 1), f32), Mrep=Mrep)

    in_maps = []
    for r in range(cfg.R):
        xloc = np.zeros((128, cfg.NLOCP), BF16)
        xloc[:, :cfg.NLOC] = x[r * cfg.NLOC:(r + 1) * cfg.NLOC].T.astype(BF16)
        m = dict(base)
        m["xTloc"] = xloc
        m.update(per_core[r])
        in_maps.append(m)

    nc = bacc.Bacc("TRN2", target_bir_lowering=False, debug=False,
                   num_devices=cfg.R)
    _build(nc, cfg, sched, has_biasA, has_biasB, has_bo)
    nc.compile()

    res = bass_utils.run_bass_kernel_spmd(
        nc, in_maps, core_ids=list(range(cfg.R)), trace=trace)
    outs = [res.results[r]["out"] for r in range(cfg.R)]
    full = np.concatenate(outs, axis=0).astype(np.float32)
    return full, res


def kernel(**inputs):
    out, _ = _run(FULL, inputs)
    return out


if __name__ == "__main__":
    pass


# revision 10
# speedup vs baseline: 1.9044x; 1.0587x over previous
"""Bass/Trainium2 kernel for nn_DirectionalGraphAttention (8 NeuronCores).

Math (see reference):
    q = (x@Wq.T + bq),  k = (x@Wk.T + bk),  v = (x@Wv.T + bv)      [N, C]
    scores[e,h] = q[row_e,h,:].k[col_e,h,:]/sqrt(HD) + ew_e
                  + (mean(x[col_e]) - mean(x[row_e])) * Wd[h] + bd[h]
    attn = softmax(scores, axis=0)            (global over ALL edges, per head)
    out[n,:] = (sum_{e: row_e==n} attn[e,h]*v[col_e,:]) @ Wo.T + bo

Strategy (8-way SPMD, one compiled program, per-core data differs):
  - Shard NODES into 8 contiguous ranges by destination; core r handles the
    edges whose row lands in its range (counts are ~E/8 by uniformity).
  - bd drops out (softmax over edges is invariant to per-head constants).
  - Per-edge source features are HOST-pregathered: for each 128-edge tile the
    host ships xgT[ch, e] = x[col_e, ch].T in bf16, packed together with the
    per-edge score bias (ew + dmean*Wd), the scatter one-hot and its
    transpose into ONE per-chunk DMA stream. No DRAM kv table, no SWDGE
    gather (the v1 design spent ~0.5 ms/core in gather descriptor ucode and
    ~5 ms of DMA-engine-seconds on 512 B gathered rows).
  - k|v are computed on the fly per tile: kv[e, 0:256] = xgT_tile.T @
    [Wk.T|Wv.T] — one 256-wide PE matmul per tile (FWL weight loads), output
    kept in PSUM (f32; skips v1's bf16 table rounding).
  - Destination rows are grouped into 128-node "slots" (sorted, padded to a
    shared compile-time schedule; pad edges have ew=-1e9 -> exp=0).
  - q[row] is never gathered: per tile, qexp = onehotT.T @ q_slot on the PE.
  - scores: DVE multiply (qexp*k) + Pool segment reduce (16-wide heads) +
    DVE bias add; exp on ACT; msgs = v*exp(scores) on DVE.
  - Unnormalized msgs are scatter-added with PE matmuls (msgs.T @ onehot)
    into PSUM (one bank per 4-slot block, single ascending pass), flushed
    to SBUF out_acc.
  - The per-head softmax denominator is AllReduced (32 B) across the 8 cores;
    out_acc is scaled by 1/Z, multiplied by Wo.T on the PE and written out.
"""

import math
import sys

sys.path.insert(0, "/opt/trn_rl_repo")

import numpy as np
import ml_dtypes

import concourse.bass as bass
import concourse.bacc as bacc
import concourse.mybir as mybir
import concourse.tile as tile
from concourse import bass_utils

BF16 = ml_dtypes.bfloat16
FP8 = ml_dtypes.float8_e4m3


# ---------------------------------------------------------------- config ----
class Config:
    def __init__(self, N=50000, E=800000, n_cores=8, chunk_tiles=16):
        assert N % n_cores == 0
        self.N, self.E, self.R = N, E, n_cores
        self.C, self.H, self.HD = 128, 8, 16
        self.NLOC = N // n_cores                       # nodes per core
        self.NSLOT = -(-self.NLOC // 128)              # 128-node slots
        self.NLOCP = self.NSLOT * 128                  # padded local nodes
        self.CT = chunk_tiles                          # tiles per chunk
        self.CE = chunk_tiles * 128                    # edges per chunk


FULL = Config()


# ------------------------------------------------------------- host prep ----
def _host_prep(cfg, x, edge_index, edge_weight, Wd_vec):
    """Shard + schedule. Returns (sched, per_core) where sched is shared
    compile-time metadata and per_core is a list of input dicts."""
    N, E, R = cfg.N, cfg.E, cfg.R
    row = np.asarray(edge_index[0], dtype=np.int64)
    col = np.asarray(edge_index[1], dtype=np.int64)
    ew = np.asarray(edge_weight, dtype=np.float32)
    xnp = np.asarray(x, dtype=np.float32)
    xm = xnp.mean(axis=1)                              # [N] row means
    dm_all = (xm[col] - xm[row]).astype(np.float32)    # per-edge dmean
    xTb = np.ascontiguousarray(xnp.T.astype(BF16))     # [128, N]

    # --- per-core edge lists, slot-sorted ---
    core_of = row // cfg.NLOC
    per_core_edges = []          # [r] -> dict s -> (cols, ews, dms, rel)
    counts = np.zeros((R, cfg.NSLOT), dtype=np.int64)
    for r in range(R):
        m = core_of == r
        rl = row[m] - r * cfg.NLOC
        cl, wl, dl = col[m], ew[m], dm_all[m]
        slot = rl // 128
        order = np.lexsort((rl, slot))
        rl, cl, wl, dl, slot = (a[order] for a in (rl, cl, wl, dl, slot))
        buckets = {}
        for s in range(cfg.NSLOT):
            sm = slot == s
            buckets[s] = (cl[sm], wl[sm], dl[sm], rl[sm] - s * 128)
            counts[r, s] = sm.sum()
        per_core_edges.append(buckets)

    # --- shared schedule: per-slot tile counts = max over cores ---
    ntile = np.maximum(1, -(-counts.max(axis=0) // 128))   # [NSLOT]
    T = int(ntile.sum())
    ntile[cfg.NSLOT - 1] += (-T) % cfg.CT                  # pad to chunk mult
    T = int(ntile.sum())
    tile_slot = []
    for s in range(cfg.NSLOT):
        tile_slot += [s] * int(ntile[s])
    nchunk = T // cfg.CT

    sched = dict(T=T, nchunk=nchunk, tile_slot=tile_slot)

    CT, CE, H = cfg.CT, cfg.CE, cfg.H
    MB = CT * H * 4                      # bias bytes per partition per chunk
    AUXW = MB + CE + CE + 2 * CE         # bias | oh | ohT | xgT

    per_core = []
    for r in range(R):
        cols = np.zeros(T * 128, dtype=np.int64)
        ews = np.full(T * 128, -1e9, dtype=np.float32)
        dms = np.zeros(T * 128, dtype=np.float32)
        rels = np.zeros(T * 128, dtype=np.int64)
        pos = 0
        for s in range(cfg.NSLOT):
            cc, wl, dl, rl = per_core_edges[r][s]
            n = len(cc)
            room = int(ntile[s]) * 128
            assert n <= room
            cols[pos:pos + n] = cc
            ews[pos:pos + n] = wl
            dms[pos:pos + n] = dl
            rels[pos:pos + n] = rl
            pos += room
        assert pos == T * 128

        # one-hot matrices [tile, e, w] and transpose, chunk layouts
        oh = np.zeros((T, 128, 128), dtype=FP8)
        ti = np.repeat(np.arange(T), 128)
        ei = np.tile(np.arange(128), T)
        oh[ti, ei, rels] = 1
        oh_c = (oh.reshape(nchunk, CT, 128, 128)
                  .transpose(0, 2, 1, 3).reshape(nchunk, 128, CE))
        ohT_c = (oh.transpose(0, 2, 1).reshape(nchunk, CT, 128, 128)
                   .transpose(0, 2, 1, 3).reshape(nchunk, 128, CE))
        # per-edge per-head score bias: ew + dmean*Wd  [T*128, H] f32
        bias_eh = (ews[:, None] + dms[:, None] * Wd_vec[None, :]).astype(
            np.float32)
        bias_c = (bias_eh.reshape(nchunk, CT, 128, H)
                  .transpose(0, 2, 1, 3).reshape(nchunk, 128, CT * H))
        # host-pregathered source features, transposed: [128ch, T*128]
        xgT = xTb[:, cols]                              # [128, T*128]
        xgT_c = xgT.reshape(128, nchunk, CE).transpose(1, 0, 2)

        aux = np.concatenate([
            np.ascontiguousarray(bias_c).view(np.uint8),
            np.ascontiguousarray(oh_c).view(np.uint8),
            np.ascontiguousarray(ohT_c).view(np.uint8),
            np.ascontiguousarray(xgT_c).view(np.uint8),
        ], axis=2)
        assert aux.shape == (nchunk, 128, AUXW)
        per_core.append(dict(meta=np.ascontiguousarray(aux)))
    return sched, per_core


# ---------------------------------------------------------- kernel build ----
def _build(nc, cfg, sched, has_biasA, has_biasB, has_bo):
    f32, bf16 = mybir.dt.float32, mybir.dt.bfloat16
    fp8 = mybir.dt.float8e4
    u8 = mybir.dt.uint8
    C, H, CT, CE = cfg.C, cfg.H, cfg.CT, cfg.CE
    NS = cfg.NSLOT
    T, nchunk = sched["T"], sched["nchunk"]
    NBLK = -(-NS // 4)           # 512-col psum blocks over slots
    MB = CT * H * 4
    AUXW = MB + CE + CE + 2 * CE

    # ---- I/O ----
    xTloc = nc.dram_tensor("xTloc", [128, cfg.NLOCP], bf16,
                           kind="ExternalInput").ap()
    W_A = nc.dram_tensor("W_A", [128, 2 * C], bf16, kind="ExternalInput").ap()
    biasA = nc.dram_tensor("biasA", [128, 2 * C], f32, kind="ExternalInput").ap()
    W_B = nc.dram_tensor("W_B", [128, C], bf16, kind="ExternalInput").ap()
    biasB = nc.dram_tensor("biasB", [128, C], f32, kind="ExternalInput").ap()
    WoT = nc.dram_tensor("WoT", [128, C], f32, kind="ExternalInput").ap()
    bo_col = nc.dram_tensor("bo_col", [128, 1], f32, kind="ExternalInput").ap()
    ones_b = nc.dram_tensor("ones_b", [128, 1], bf16, kind="ExternalInput").ap()
    Mrep = nc.dram_tensor("Mrep", [cfg.H, 128], f32, kind="ExternalInput").ap()
    Msum = nc.dram_tensor("Msum", [128, cfg.H], f32, kind="ExternalInput").ap()
    meta_d = nc.dram_tensor("meta", [nchunk, 128, AUXW], u8,
                            kind="ExternalInput").ap()
    outT = nc.dram_tensor("outT", [128, NS * 128], f32,
                          kind="ExternalOutput").ap()

    with tile.TileContext(nc) as tc:
        with (
            tc.tile_pool(name="persist", bufs=1) as pp,
            tc.tile_pool(name="wpool", bufs=1) as wp,
            tc.tile_pool(name="io", bufs=3) as iop,
            tc.tile_pool(name="psK", bufs=2, space="PSUM") as psK,
            tc.tile_pool(name="psQ", bufs=2, space="PSUM") as psQ,
            tc.tile_pool(name="psS", bufs=1, space="PSUM") as psS,
            tc.tile_pool(name="psZ", bufs=1, space="PSUM") as psZ,
            tc.tile_pool(name="work", bufs=3) as wk,
        ):
            # persistent SBUF
            qlocal = pp.tile([128, NS * 128], bf16, tag="qlocal")
            # float32r so the finale's 512-col 1-cyc/row matmuls can consume
            # out_acc directly (BIR requires producers to round to FP32r)
            out_acc = pp.tile([128, NS * 128], mybir.dt.float32r,
                              tag="out_acc")

            # weights in SBUF
            WA_sb = wp.tile([128, 2 * C], bf16, tag="WA")
            nc.sync.dma_start(WA_sb[:], W_A[:])
            WB_sb = wp.tile([128, C], bf16, tag="WB")
            nc.sync.dma_start(WB_sb[:], W_B[:])
            WoT_sb = wp.tile([128, C], f32, tag="WoT")
            nc.sync.dma_start(WoT_sb[:], WoT[:])
            bo_sb = wp.tile([128, 1], f32, tag="bo")
            nc.sync.dma_start(bo_sb[:], bo_col[:])
            ones_sb = wp.tile([128, 1], bf16, tag="ones")
            nc.sync.dma_start(ones_sb[:], ones_b[:])
            Mrep_sb = wp.tile([cfg.H, 128], f32, tag="Mrep")
            nc.sync.dma_start(Mrep_sb[:], Mrep[:])
            Msum_sb = wp.tile([128, cfg.H], f32, tag="Msum")
            nc.sync.dma_start(Msum_sb[:], Msum[:])
            if has_biasA:
                bA_sb = wp.tile([128, 2 * C], f32, tag="bA")
                nc.sync.dma_start(bA_sb[:], biasA[:])
            if has_biasB:
                bB_sb = wp.tile([128, C], f32, tag="bB")
                nc.sync.dma_start(bB_sb[:], biasB[:])

            # ------------- P2: local q tilde -> SBUF qlocal -------------
            s = 0
            while s < NS:
                nb = min(4, NS - s)
                xt4 = iop.tile([128, 512], bf16, tag="xt2")
                nc.scalar.dma_start(xt4[:, 0:nb * 128],
                                    xTloc[:, s * 128:(s + nb) * 128])
                for j in range(nb):
                    ps = psQ.tile([128, 512], f32, tag="qexp")
                    nc.tensor.matmul(out=ps[:, 0:C],
                                     lhsT=xt4[:, j * 128:(j + 1) * 128],
                                     rhs=WB_sb[:], start=True, stop=True)
                    dstq = qlocal[:, (s + j) * 128:(s + j + 1) * 128]
                    if has_biasB:
                        nc.vector.tensor_tensor(out=dstq, in0=ps[:, 0:C],
                                                in1=bB_sb[:],
                                                op=mybir.AluOpType.add)
                    elif (s + j) % 2 == 0:
                        nc.scalar.copy(dstq, ps[:, 0:C])
                    else:
                        nc.vector.tensor_copy(dstq, ps[:, 0:C])
                s += nb

            # ---------------- P3: edge chunks ----------------
            ngroup = CT // 4
            blk_seen = [False] * NBLK
            st = dict(scat_ps=None, cur_blk=-1)

            def flush_block(blk, ps_tile):
                lo, hi_ = blk * 4, min(blk * 4 + 4, NS)
                w = (hi_ - lo) * 128
                dst = out_acc[:, blk * 512: blk * 512 + w]
                if blk_seen[blk]:
                    nc.vector.tensor_tensor(out=dst, in0=dst,
                                            in1=ps_tile[:, 0:w],
                                            op=mybir.AluOpType.add)
                else:
                    nc.vector.tensor_copy(dst, ps_tile[:, 0:w])
                    blk_seen[blk] = True

            def issue_scatter(kind, gt, msgs_t, j, oh_ap):
                if kind == "z":
                    issue_z(msgs_t)
                    return
                sl = sched["tile_slot"][gt]
                blk = sl // 4
                if blk != st["cur_blk"]:
                    if st["scat_ps"] is not None:
                        flush_block(st["cur_blk"], st["scat_ps"])
                    st["scat_ps"] = psS.tile([128, 512], f32, tag="scat",
                                             name="scat_ps")
                    # PSUM accumulation groups cannot survive same-bank
                    # interleaving (HW): pre-zero the bank and use
                    # start=False on every matmul instead.
                    nc.vector.memset(st["scat_ps"][:], 0.0)
                    st["cur_blk"] = blk
                nc.tensor.matmul(
                    out=st["scat_ps"][:, (sl % 4) * 128:(sl % 4) * 128 + 128],
                    lhsT=msgs_t[:, j * C:(j + 1) * C],
                    rhs=oh_ap,
                    start=False, stop=True, skip_group_check=True)

            # softmax denominator: one ones-matmul per chunk accumulating
            # into a pinned PSUM column across all of P3
            zps = psZ.tile([128, 2], f32, tag="zps")
            nc.vector.memset(zps[:], 0.0)

            def issue_z(exps_t):
                nc.tensor.matmul(out=zps[:, 0:1], lhsT=exps_t[:],
                                 rhs=ones_sb[:], start=False, stop=True,
                                 skip_group_check=True)

            # scatter matmuls are issued PD tiles behind their msgs so the
            # in-order PE queue never blocks on the DVE/ACT score chain
            PD = 8
            pending = []
            PF = 2                   # aux DMA prefetch depth (chunks)
            aux_tiles = {}

            def load_aux(ci):
                a = wk.tile([128, AUXW], u8, tag="aux", bufs=4)
                eng = nc.sync if ci % 2 == 0 else nc.scalar
                eng.dma_start(a[:], meta_d[ci, :, :])
                aux_tiles[ci] = a

            for ci in range(min(PF + 1, nchunk)):
                load_aux(ci)

            for c in range(nchunk):
                if c + PF + 1 < nchunk:
                    load_aux(c + PF + 1)
                aux = aux_tiles.pop(c)
                bias_sb = aux[:, 0:MB].bitcast(f32)            # [128, CT*H]
                oh_sb = aux[:, MB:MB + CE].bitcast(fp8)
                ohT_sb = aux[:, MB + CE:MB + 2 * CE].bitcast(fp8)
                xgT_sb = aux[:, MB + 2 * CE:].bitcast(bf16)    # [128, CE]

                sc_buf = wk.tile([128, CT * H], f32, tag="scb")
                kvsbs = []
                for g in range(ngroup):
                    kvps = psK.tile([128, 4, 2 * C], f32, tag="kv")
                    qps = psQ.tile([128, 512], f32, tag="qexp")
                    for j in range(4):
                        t = g * 4 + j
                        nc.tensor.matmul(
                            out=kvps[:, j, :],
                            lhsT=xgT_sb[:, t * 128:(t + 1) * 128],
                            rhs=WA_sb[:], start=True, stop=True)
                    for j in range(4):
                        t = g * 4 + j
                        sl = sched["tile_slot"][c * CT + t]
                        nc.tensor.matmul(
                            out=qps[:, j * 128:(j + 1) * 128],
                            lhsT=ohT_sb[:, t * 128:(t + 1) * 128],
                            rhs=qlocal[:, sl * 128:(sl + 1) * 128],
                            start=True, stop=True)
                    while len(pending) > PD - 4:
                        issue_scatter(*pending.pop(0))
                    # kv -> SBUF bf16 in one hop (frees the PSUM bank fast;
                    # also enables 16-bit-rate engines downstream)
                    kvsb = wk.tile([128, 4, 2 * C], bf16, tag="kvsb", bufs=6)
                    if has_biasA:
                        nc.vector.tensor_tensor(
                            out=kvsb[:], in0=kvps[:],
                            in1=bA_sb[:].rearrange("p c -> p () c")
                                        .to_broadcast([128, 4, 2 * C]),
                            op=mybir.AluOpType.add)
                    else:
                        nc.scalar.copy(kvsb[:], kvps[:])
                    kvsbs.append(kvsb)
                    prod = wk.tile([128, 512], bf16, tag="prod")
                    nc.vector.tensor_tensor(
                        out=prod[:].rearrange("p (t c) -> p t c", t=4, c=C),
                        in0=qps[:].rearrange("p (t c) -> p t c", t=4, c=C),
                        in1=kvsb[:, :, 0:C],
                        op=mybir.AluOpType.mult)
                    nc.vector.tensor_reduce(
                        out=sc_buf[:, g * 4 * H:(g + 1) * 4 * H],
                        in_=prod[:].rearrange("p (t h d) -> p t h d",
                                              t=4, h=H, d=cfg.HD),
                        axis=mybir.AxisListType.X, op=mybir.AluOpType.add)
                # chunk-batched bias + exp
                nc.gpsimd.tensor_tensor(
                    out=sc_buf[:], in0=sc_buf[:], in1=bias_sb,
                    op=mybir.AluOpType.add)
                exps = wk.tile([128, CT * H], bf16, tag="exps")
                nc.scalar.activation(exps[:], sc_buf[:],
                                     mybir.ActivationFunctionType.Exp)
                for g in range(ngroup):
                    exps_g = exps[:, g * 4 * H:(g + 1) * 4 * H]
                    msgs = wk.tile([128, 512], bf16, tag="msgs", bufs=4)
                    nc.gpsimd.tensor_tensor(
                        out=msgs[:].rearrange("p (t h d) -> p t h d",
                                              t=4, h=H, d=cfg.HD),
                        in0=kvsbs[g][:, :, C:2 * C].rearrange(
                            "p t (h d) -> p t h d", h=H, d=cfg.HD),
                        in1=exps_g.rearrange("p (t h) -> p t h ()",
                                             t=4, h=H)
                                  .to_broadcast([128, 4, H, cfg.HD]),
                        op=mybir.AluOpType.mult)
                    for j in range(4):
                        t = g * 4 + j
                        pending.append(("s", c * CT + t, msgs, j,
                                        oh_sb[:, t * 128:(t + 1) * 128]))
                pending.append(("z", 0, exps, 0, None))
            while pending:
                issue_scatter(*pending.pop(0))
            flush_block(st["cur_blk"], st["scat_ps"])

            # ---------------- P4: finale ----------------
            # per-(tile,head) partial Z -> per-head Z -> AllReduce -> 1/Z
            zaccSB = wk.tile([128, 1], f32, tag="zaccSB")
            nc.vector.tensor_copy(zaccSB[:], zps[:, 0:1])
            zsum_ps = psQ.tile([128, 512], f32, tag="qexp")
            nc.tensor.matmul(out=zsum_ps[0:cfg.H, 0:1], lhsT=Msum_sb[:],
                             rhs=zaccSB[:], start=True, stop=True)
            zsb = wk.tile([cfg.H, 1], f32, tag="zsb")
            nc.vector.tensor_copy(zsb[:], zsum_ps[0:cfg.H, 0:1])
            with tc.tile_pool(name="dram", bufs=1, space="DRAM") as dp:
                zin_d = dp.tile([cfg.H, 1], f32)
                zout_d = dp.tile([cfg.H, 1], f32)
                nc.sync.dma_start(zin_d[:], zsb[:])
                nc.gpsimd.collective_compute(
                    "AllReduce", mybir.AluOpType.add,
                    replica_groups=[list(range(cfg.R))],
                    ins=[zin_d.opt()], outs=[zout_d.opt()])
                zvec = wk.tile([cfg.H, 1], f32, tag="zvec")
                nc.sync.dma_start(zvec[:], zout_d[:])
            zcol_ps = psQ.tile([128, 512], f32, tag="qexp")
            nc.tensor.matmul(out=zcol_ps[:, 0:1], lhsT=Mrep_sb[:], rhs=zvec[:],
                             start=True, stop=True)
            rz = wk.tile([128, 1], f32, tag="rz")
            nc.vector.reciprocal(rz[:], zcol_ps[:, 0:1])
            # fold 1/Z into WoT's contraction rows: out = (WoT*rz).T @ acc
            WoT_rz = wk.tile([128, C], mybir.dt.float32r, tag="WoT_rz")
            nc.vector.tensor_scalar(out=WoT_rz[:], in0=WoT_sb[:],
                                    scalar1=rz[:], scalar2=None,
                                    op0=mybir.AluOpType.mult)
            NW = NS * 128 // 512      # 512-col output windows
            for w in range(NW + (1 if NS * 128 % 512 else 0)):
                cols = min(512, NS * 128 - w * 512)
                ps = psQ.tile([128, 512], f32, tag="qexp")
                nc.tensor.matmul(
                    out=ps[:, 0:cols],
                    lhsT=WoT_rz[:],
                    rhs=out_acc[:, w * 512:w * 512 + cols],
                    start=True, stop=True)
                of = iop.tile([128, 512], f32, tag="of")
                if has_bo:
                    nc.vector.tensor_scalar(out=of[:, 0:cols],
                                            in0=ps[:, 0:cols],
                                            scalar1=bo_sb[:], scalar2=None,
                                            op0=mybir.AluOpType.add)
                elif w % 2 == 0:
                    nc.vector.tensor_copy(of[:, 0:cols], ps[:, 0:cols])
                else:
                    nc.scalar.copy(of[:, 0:cols], ps[:, 0:cols])
                nc.sync.dma_start(outT[:, w * 512:w * 512 + cols],
                                  of[:, 0:cols])
    return nc


# -------------------------------------------------------------- frontend ----
def _run(cfg, inputs, trace=False):
    x = np.asarray(inputs["x"], dtype=np.float32)
    sched, per_core = _host_prep(cfg, x, inputs["edge_index"],
                                 inputs["edge_weight"],
                                 np.asarray(inputs["Wd"],
                                            np.float32).reshape(-1))

    f32 = np.float32
    Wq = np.asarray(inputs["Wq"], f32); bq = np.asarray(inputs["bq"], f32)
    Wk = np.asarray(inputs["Wk"], f32); bk = np.asarray(inputs["bk"], f32)
    Wv = np.asarray(inputs["Wv"], f32); bv = np.asarray(inputs["bv"], f32)
    Wo = np.asarray(inputs["Wo"], f32); bo = np.asarray(inputs["bo"], f32)
    inv = 1.0 / math.sqrt(cfg.HD)

    W_A = np.concatenate([Wk.T, Wv.T], axis=1).astype(BF16)   # [128, 256]
    biasA = np.tile(np.concatenate([bk, bv])[None, :], (128, 1))
    W_B = (Wq.T * inv).astype(BF16)
    biasB = np.tile((bq * inv)[None, :], (128, 1))
    has_biasA = bool(np.any(biasA)); has_biasB = bool(np.any(biasB))
    has_bo = bool(np.any(bo))
    Mrep = np.zeros((cfg.H, 128), f32)
    for h in range(cfg.H):
        Mrep[h, h * 16:(h + 1) * 16] = 1.0
    Msum = np.zeros((128, cfg.H), f32)
    for t in range(128 // cfg.H):
        for h in range(cfg.H):
            Msum[t * cfg.H + h, h] = 1.0

    base = dict(
        W_A=W_A, biasA=biasA.astype(f32), W_B=W_B,
        biasB=biasB.astype(f32), WoT=np.ascontiguousarray(Wo.T),
        bo_col=bo.reshape(cfg.C, 1).astype(f32),
        ones_b=np.ones((128, 1), BF16), Mrep=Mrep, Msum=Msum)

    in_maps = []
    for r in range(cfg.R):
        xloc = np.zeros((128, cfg.NLOCP), BF16)
        xloc[:, :cfg.NLOC] = x[r * cfg.NLOC:(r + 1) * cfg.NLOC].T.astype(BF16)
        m = dict(base)
        m["xTloc"] = xloc
        m.update(per_core[r])
        in_maps.append(m)

    nc = bacc.Bacc("TRN2", target_bir_lowering=False, debug=False,
                   num_devices=cfg.R)
    _build(nc, cfg, sched, has_biasA, has_biasB, has_bo)
    nc.compile()

    res = bass_utils.run_bass_kernel_spmd(
        nc, in_maps, core_ids=list(range(cfg.R)), trace=trace)
    outs = [np.asarray(res.results[r]["outT"])[:, :cfg.NLOC].T
            for r in range(cfg.R)]
    full = np.concatenate(outs, axis=0).astype(np.float32)
    return full, res


def kernel(**inputs):
    out, _ = _run(FULL, inputs)
    return out


if __name__ == "__main__":
    pass
